# revision 1
# baseline (speedup 1.0000x reference)
"""Trainium2 Bass kernel for CLRNet SimOTA-style assignment (B=8, N=4096, M=32, K=72).

Strategy: pure data-parallel over batch — one batch element per NeuronCore.

Math notes (vs the jax reference):
  * The masked L1 distance D[n,m] = sum_k v[m,k] * |pred_x[n,k] - tgt_x[m,k]| is the
    only O(N*M*K) quantity needed.  Using t2 = v*t and pred_x >= 0:
        sum_k |p - t2| = D + sum_k p * (1 - v)
    so D = abs-reduce(p - t2) - C with C = pred_x @ (1-v)^T computed on the
    TensorEngine (contraction K=72).
  * line IOU: ovr_k = 2L - |d_k|, union_k = 2L + |d_k|  (L=15), both masked, so
        iou[n,m] = (30*len[m] - D[n,m]) / (30*len[m] + D[n,m] + 1e-9)
    which is monotone decreasing in D -> top-4 ious per m == 4 smallest D per m.
  * dynamic_ks = clip(floor(sum top4 clamped ious), 1, 4); matching is recovered
    value-wise: match[n,m] = (cost[n,m] <= ks-th smallest cost of column m) & mask.
  * DVE `max` (top-8 per partition) gives the 4 smallest D / costs per m after a
    PE transpose with scale=-1.
"""

import os
import sys

sys.path.insert(0, "/opt/trn_rl_repo")

import numpy as np

B, N, M, K = 8, 4096, 32, 72
D_FEAT = 78
IMG_W = 800.0
BIG = 100000.0
BIG2 = 100000.0  # index-packing constant
BIGINF = 1e30
EPS = 1e-12

_CACHE = {}


def _build_nc(n=N, reps=1):
    import concourse.bass as bass
    import concourse.bacc as bacc
    import concourse.mybir as mybir
    from concourse.tile import TileContext

    Alu = mybir.AluOpType
    ACT = mybir.ActivationFunctionType
    dt = mybir.dt
    X = mybir.AxisListType.X
    XY = mybir.AxisListType.XY

    P = 128
    T = n // P  # number of n-tiles
    TM = T * M

    nc = bacc.Bacc()

    preds = nc.declare_dram_parameter("preds", [n, D_FEAT], dt.float32, isOutput=False)
    targets = nc.declare_dram_parameter("targets", [M, D_FEAT], dt.float32, isOutput=False)
    maskf = nc.declare_dram_parameter("maskf", [M], dt.float32, isOutput=False)
    out_assigned = nc.declare_dram_parameter("out_assigned", [n], dt.int32, isOutput=True)
    out_matched = nc.declare_dram_parameter("out_matched", [n], dt.int32, isOutput=True)

    scr_t2 = nc.dram_tensor("scr_t2", [M, K], dt.float32)
    scr_small = nc.dram_tensor("scr_small", [8, M], dt.float32)
    scr_thr = nc.dram_tensor("scr_thr", [M, 1], dt.float32)
    scr_g = nc.dram_tensor("scr_g", [3, 1], dt.float32)

    with TileContext(nc) as tc:
        with (
            tc.tile_pool(name="const", bufs=1) as cpool,
            tc.tile_pool(name="sb", bufs=1) as pool,
            tc.tile_pool(name="diffp", bufs=6) as diffp,
            tc.tile_pool(name="ps", bufs=2, space="PSUM") as psum,
            tc.tile_pool(name="pscx", bufs=1, space="PSUM") as pscx,
        ):
            f32 = dt.float32

            # ---------------- constants ----------------
            icol = cpool.tile([P, 1], f32)
            nc.gpsimd.iota(icol[:], pattern=[[0, 1]], channel_multiplier=1,
                           allow_small_or_imprecise_dtypes=True)
            irow = cpool.tile([P, P], f32)
            nc.gpsimd.iota(irow[:], pattern=[[1, P]], channel_multiplier=0,
                           allow_small_or_imprecise_dtypes=True)
            ident = cpool.tile([P, P], f32)
            nc.vector.tensor_scalar(ident[:], irow[:], icol[:], None, Alu.is_equal)
            # BMI[p, m] = BIG2 - m
            bmi = cpool.tile([P, M], f32)
            nc.gpsimd.iota(bmi[:], pattern=[[-1, M]], base=int(BIG2),
                           channel_multiplier=0, allow_small_or_imprecise_dtypes=True)
            # iota4[p, j] = j + 1
            iota4 = cpool.tile([P, 4], f32)
            nc.gpsimd.iota(iota4[:], pattern=[[1, 4]], base=1, channel_multiplier=0,
                           allow_small_or_imprecise_dtypes=True)

            # ---------------- body (emitted `reps` times for benchmarking) ----------------
            def _body():

                P_sb = pool.tile([P, T * D_FEAT], f32)
                pview = preds[:].rearrange("(t p) d -> p t d", p=P)
                tch = max(1, T // 4)
                for t0 in range(0, T, tch):
                    t1 = min(T, t0 + tch)
                    nc.sync.dma_start(
                        out=P_sb[:].rearrange("p (t d) -> p t d", d=D_FEAT)[:, t0:t1],
                        in_=pview[:, t0:t1],
                    )
                T_sb = pool.tile([M, D_FEAT], f32)
                nc.sync.dma_start(out=T_sb[:], in_=targets[:])
                Mk = pool.tile([M, 1], f32)
                nc.sync.dma_start(out=Mk[:], in_=maskf[:].unsqueeze(1))

                # ---------------- target-side precompute (M-partition layout) ----------
                tdx = T_sb[:, 6:78]
                v0 = pool.tile([M, K], f32)
                nc.vector.tensor_scalar(v0[:], tdx, 0.0, None, Alu.is_ge)
                v1 = pool.tile([M, K], f32)
                nc.vector.tensor_scalar(v1[:], tdx, IMG_W, None, Alu.is_lt)
                vv = pool.tile([M, K], f32)
                nc.vector.tensor_tensor(vv[:], v0[:], v1[:], Alu.mult)
                t2 = pool.tile([M, K], f32)
                nc.vector.tensor_tensor(t2[:], tdx, vv[:], Alu.mult)
                onemv = pool.tile([M, K], f32)
                nc.vector.tensor_scalar(onemv[:], vv[:], -1.0, 1.0, Alu.mult, Alu.add)
                lenr = pool.tile([M, 1], f32)
                nc.vector.tensor_reduce(lenr[:], vv[:], axis=X, op=Alu.add)
                lenc = pool.tile([M, 1], f32)
                nc.vector.tensor_scalar(lenc[:], lenr[:], 1.0, None, Alu.max)
                invlen = pool.tile([M, 1], f32)
                nc.vector.reciprocal(invlen[:], lenc[:])
                a30 = pool.tile([M, 1], f32)
                nc.vector.tensor_scalar(a30[:], lenr[:], 30.0, None, Alu.mult)
                aeps = pool.tile([M, 1], f32)
                nc.vector.tensor_scalar(aeps[:], a30[:], 1e-9, None, Alu.add)
                bigoff = pool.tile([M, 1], f32)
                nc.vector.tensor_scalar(bigoff[:], Mk[:], -BIG, BIG, Alu.mult, Alu.add)

                # t2 row -> broadcast to all partitions
                nc.sync.dma_start(out=scr_t2[:], in_=t2[:])
                TgtRep = pool.tile([P, M * K], f32)
                nc.sync.dma_start(
                    out=TgtRep[:],
                    in_=scr_t2[:].flatten().unsqueeze(0).broadcast_to([P, M * K]),
                )

                # small per-m rows -> SRep [128, (i, m)]
                spack = pool.tile([M, 8], f32)
                nc.vector.tensor_copy(spack[:, 0:1], T_sb[:, 2:3])   # tx
                nc.vector.tensor_copy(spack[:, 1:2], T_sb[:, 3:4])   # ty
                nc.vector.tensor_copy(spack[:, 2:3], T_sb[:, 4:5])   # tth
                nc.vector.tensor_copy(spack[:, 3:4], T_sb[:, 1:2])   # label
                nc.vector.tensor_copy(spack[:, 4:5], invlen[:])
                nc.vector.tensor_copy(spack[:, 5:6], Mk[:])
                nc.vector.tensor_copy(spack[:, 6:7], bigoff[:])
                nc.vector.tensor_copy(spack[:, 7:8], Mk[:])
                nc.sync.dma_start(out=scr_small[:].rearrange("i m -> m i"), in_=spack[:])
                SRep = pool.tile([P, 8 * M], f32)
                nc.sync.dma_start(
                    out=SRep[:],
                    in_=scr_small[:].flatten().unsqueeze(0).broadcast_to([P, 8 * M]),
                )

                def srep(i):
                    return SRep[:, i * M:(i + 1) * M].unsqueeze(1).broadcast_to([P, T, M])

                # onemv transposed [K, M] for the C matmul
                onemvT_ps = pscx.tile([K, M], f32)
                nc.tensor.transpose(onemvT_ps[:], onemv[:], ident[0:M, 0:M])
                onemvT = pool.tile([K, M], f32)
                nc.scalar.activation(onemvT[:], onemvT_ps[:], ACT.Copy)

                # predxT [72, n] via PE transposes
                predxT = pool.tile([K, n], f32)
                Pv = P_sb[:].rearrange("p (t d) -> p t d", d=D_FEAT)
                for t in range(T):
                    tp = psum.tile([K, P], f32, tag="tp")
                    nc.tensor.transpose(tp[:], Pv[:, t, 6:78], ident[:])
                    nc.scalar.activation(predxT[:, t * P:(t + 1) * P], tp[:], ACT.Copy)

                # ---------------- heavy stage: Draw + C ----------------
                Draw = pool.tile([P, TM], f32)
                TgtRep3 = TgtRep[:].rearrange("p (m k) -> p m k", k=K)
                Cps = pscx.tile([P, TM], f32)
                # subtracts run on GPSIMD (Pool) so they overlap the DVE reduces.
                # For the last `n_fold` tiles the |.| is taken on ACT and the k-
                # reduction is mostly folded on GPSIMD (72->36->18->9 adds), leaving
                # DVE only a 9-wide reduce -- this balances DVE vs Pool vs ACT.
                n_fold = (T * 9) // 32
                for t in range(T):
                    pb = Pv[:, t, 6:78].unsqueeze(1).broadcast_to([P, M, K])
                    diff = diffp.tile([P, M * K], f32, tag="diff")
                    dv = diff[:].rearrange("p (m k) -> p m k", k=K)
                    nc.gpsimd.tensor_tensor(dv, pb, TgtRep3, Alu.subtract)
                    if t >= T - n_fold:
                        nc.scalar.activation(diff[:], diff[:], ACT.Abs)
                        nc.gpsimd.tensor_tensor(
                            dv[:, :, 0:36], dv[:, :, 0:36], dv[:, :, 36:72], Alu.add)
                        nc.gpsimd.tensor_tensor(
                            dv[:, :, 0:18], dv[:, :, 0:18], dv[:, :, 18:36], Alu.add)
                        nc.gpsimd.tensor_tensor(
                            dv[:, :, 0:9], dv[:, :, 0:9], dv[:, :, 9:18], Alu.add)
                        nc.vector.tensor_reduce(
                            Draw[:, t * M:(t + 1) * M], dv[:, :, 0:9],
                            axis=X, op=Alu.add,
                        )
                    else:
                        nc.vector.tensor_reduce(
                            Draw[:, t * M:(t + 1) * M], dv,
                            axis=X, op=Alu.add, apply_absolute_value=True,
                        )
                    nc.tensor.matmul(
                        Cps[:, t * M:(t + 1) * M],
                        predxT[:, t * P:(t + 1) * P], onemvT[:],
                        start=True, stop=True,
                    )

                Dm = pool.tile([P, TM], f32)
                nc.vector.tensor_tensor(Dm[:], Draw[:], Cps[:], Alu.subtract)

                # ---------------- phase 2: cost assembly ----------------
                d3 = lambda ap: ap.rearrange("p (t m) -> p t m", m=M)

                dist = pool.tile([P, TM], f32)
                nc.gpsimd.tensor_tensor(d3(dist[:]), d3(Dm[:]), srep(4), Alu.mult)

                def pcol(c):
                    return Pv[:, :, c].unsqueeze(2).broadcast_to([P, T, M])

                dxf = pool.tile([P, TM], f32)
                nc.gpsimd.tensor_tensor(d3(dxf[:]), pcol(2), srep(0), Alu.subtract)
                dyf = pool.tile([P, TM], f32)
                nc.gpsimd.tensor_tensor(d3(dyf[:]), pcol(3), srep(1), Alu.subtract)
                nc.scalar.activation(dxf[:], dxf[:], ACT.Square)
                nc.scalar.activation(dyf[:], dyf[:], ACT.Square)
                xyf = pool.tile([P, TM], f32)
                nc.gpsimd.tensor_tensor(xyf[:], dxf[:], dyf[:], Alu.add)
                nc.scalar.activation(xyf[:], xyf[:], ACT.Sqrt)
                thf = pool.tile([P, TM], f32)
                nc.gpsimd.tensor_tensor(d3(thf[:]), pcol(4), srep(2), Alu.subtract)
                nc.scalar.activation(thf[:], thf[:], ACT.Abs)

                # global maxes of dist / xyf / thf
                mx3 = pool.tile([P, 3], f32)
                nc.vector.tensor_reduce(mx3[:, 0:1], d3(dist[:]), axis=XY, op=Alu.max)
                nc.vector.tensor_reduce(mx3[:, 1:2], d3(xyf[:]), axis=XY, op=Alu.max)
                nc.vector.tensor_reduce(mx3[:, 2:3], d3(thf[:]), axis=XY, op=Alu.max)
                mxT_ps = psum.tile([3, P], f32, tag="tp")
                nc.tensor.transpose(mxT_ps[:], mx3[:], ident[:])
                mxT = pool.tile([3, P], f32)
                nc.scalar.activation(mxT[:], mxT_ps[:], ACT.Copy)
                g3 = pool.tile([3, 1], f32)
                nc.vector.tensor_reduce(g3[:], mxT[:], axis=X, op=Alu.max)
                nc.sync.dma_start(out=scr_g[:], in_=g3[:])
                gmx = pool.tile([P, 3], f32)
                nc.sync.dma_start(
                    out=gmx[:], in_=scr_g[:].flatten().unsqueeze(0).broadcast_to([P, 3])
                )
                gmx2 = pool.tile([P, 3], f32)
                nc.vector.tensor_scalar(gmx2[:], gmx[:], 1e-6, None, Alu.max)
                ginv = pool.tile([P, 3], f32)
                nc.vector.reciprocal(ginv[:], gmx2[:])
                nginv = pool.tile([P, 3], f32)
                nc.vector.tensor_scalar(nginv[:], ginv[:], -1.0, None, Alu.mult)

                # focal
                lg = P_sb[:].rearrange("p (t d) -> p t d", d=D_FEAT)[:, :, 0:2]
                sig = pool.tile([P, T * 2], f32)
                nc.scalar.activation(sig[:].rearrange("p (t c) -> p t c", c=2), lg, ACT.Sigmoid)
                qq = pool.tile([P, T * 2], f32)
                nc.vector.tensor_scalar(qq[:], sig[:], -1.0, 1.0, Alu.mult, Alu.add)
                epsc = cpool.tile([P, 1], f32)
                nc.gpsimd.memset(epsc[:], EPS)
                lp = pool.tile([P, T * 2], f32)
                nc.scalar.activation(lp[:], sig[:], ACT.Ln, bias=epsc[:])
                lq = pool.tile([P, T * 2], f32)
                nc.scalar.activation(lq[:], qq[:], ACT.Ln, bias=epsc[:])
                p2 = pool.tile([P, T * 2], f32)
                nc.vector.tensor_tensor(p2[:], sig[:], sig[:], Alu.mult)
                q2 = pool.tile([P, T * 2], f32)
                nc.vector.tensor_tensor(q2[:], qq[:], qq[:], Alu.mult)
                pos = pool.tile([P, T * 2], f32)
                nc.vector.scalar_tensor_tensor(pos[:], lp[:], -0.25, q2[:], Alu.mult, Alu.mult)
                neg = pool.tile([P, T * 2], f32)
                nc.vector.scalar_tensor_tensor(neg[:], lq[:], -0.75, p2[:], Alu.mult, Alu.mult)
                fdiff = pool.tile([P, T * 2], f32)
                nc.vector.tensor_tensor(fdiff[:], pos[:], neg[:], Alu.subtract)
                fv = fdiff[:].rearrange("p (t c) -> p t c", c=2)
                d0b = fv[:, :, 0].unsqueeze(2).broadcast_to([P, T, M])
                ddt = pool.tile([P, T], f32)
                nc.vector.tensor_tensor(ddt[:], fv[:, :, 1], fv[:, :, 0], Alu.subtract)
                ddb = ddt[:].unsqueeze(2).broadcast_to([P, T, M])
                cls = pool.tile([P, TM], f32)
                nc.gpsimd.tensor_tensor(d3(cls[:]), srep(3), ddb, Alu.mult)
                nc.gpsimd.tensor_tensor(d3(cls[:]), d3(cls[:]), d0b, Alu.add)

                # scores
                ds_ = pool.tile([P, TM], f32)
                nc.scalar.activation(ds_[:], dist[:], ACT.Copy, bias=1.01, scale=nginv[:, 0:1])
                xys = pool.tile([P, TM], f32)
                nc.scalar.activation(xys[:], xyf[:], ACT.Copy, bias=1.01, scale=nginv[:, 1:2])
                ths = pool.tile([P, TM], f32)
                nc.scalar.activation(ths[:], thf[:], ACT.Copy, bias=1.01, scale=nginv[:, 2:3])
                s3 = pool.tile([P, TM], f32)
                nc.gpsimd.tensor_tensor(s3[:], ds_[:], xys[:], Alu.mult)
                nc.gpsimd.tensor_tensor(s3[:], s3[:], ths[:], Alu.mult)
                sq = pool.tile([P, TM], f32)
                nc.scalar.activation(sq[:], s3[:], ACT.Square)
                cost = pool.tile([P, TM], f32)
                nc.vector.scalar_tensor_tensor(cost[:], sq[:], -3.0, cls[:], Alu.mult, Alu.add)
                nc.gpsimd.tensor_tensor(d3(cost[:]), d3(cost[:]), srep(5), Alu.mult)
                nc.gpsimd.tensor_tensor(d3(cost[:]), d3(cost[:]), srep(6), Alu.add)

                # ---------------- transposes (negated) ----------------
                costTn = pool.tile([M, n], f32)
                DTn = pool.tile([M, n], f32)
                cv = cost[:].rearrange("p (t m) -> p t m", m=M)
                dv = Dm[:].rearrange("p (t m) -> p t m", m=M)
                for src, dst in ((cv, costTn), (dv, DTn)):
                    for g in range(T // 4):
                        tpg = psum.tile([M, 4 * P], f32, tag="tpT")
                        for j in range(4):
                            t = 4 * g + j
                            nc.tensor.transpose(tpg[:, j * P:(j + 1) * P], src[:, t, :], ident[:])
                        nc.scalar.activation(dst[:, g * 4 * P:(g + 1) * 4 * P], tpg[:],
                                             ACT.Copy, scale=-1.0)

                # ---------------- top-k ----------------
                c8 = pool.tile([M, 8], f32)
                nc.vector.max(c8[:], costTn[:])
                d8 = pool.tile([M, 8], f32)
                nc.vector.max(d8[:], DTn[:])

                # dynamic ks from 4 smallest D (d8[:, :4] = -d, descending)
                num4 = pool.tile([M, 4], f32)
                nc.vector.tensor_scalar(num4[:], d8[:, 0:4], a30[:], None, Alu.add)  # 30L - d
                den4 = pool.tile([M, 4], f32)
                nc.vector.tensor_scalar(den4[:], d8[:, 0:4], -1.0, None, Alu.mult)
                nc.vector.tensor_scalar(den4[:], den4[:], aeps[:], None, Alu.add)     # 30L + d + eps
                rec4 = pool.tile([M, 4], f32)
                nc.vector.reciprocal(rec4[:], den4[:])
                iou4 = pool.tile([M, 4], f32)
                nc.vector.tensor_tensor(iou4[:], num4[:], rec4[:], Alu.mult)
                nc.vector.tensor_scalar(iou4[:], iou4[:], Mk[:], 0.0, Alu.mult, Alu.max)
                S4 = pool.tile([M, 1], f32)
                nc.vector.tensor_reduce(S4[:], iou4[:], axis=X, op=Alu.add)
                # ks = clip(floor(S), 1, 4) = 1 + [S>=2] + [S>=3] + [S>=4]  (S in [0,4])
                ge2 = pool.tile([M, 1], f32)
                nc.vector.tensor_scalar(ge2[:], S4[:], 2.0, None, Alu.is_ge)
                ge3 = pool.tile([M, 1], f32)
                nc.vector.tensor_scalar(ge3[:], S4[:], 3.0, None, Alu.is_ge)
                ks = pool.tile([M, 1], f32)
                nc.vector.tensor_scalar(ks[:], S4[:], 4.0, None, Alu.is_ge)
                nc.vector.tensor_tensor(ks[:], ks[:], ge2[:], Alu.add)
                nc.vector.tensor_tensor(ks[:], ks[:], ge3[:], Alu.add)
                nc.vector.tensor_scalar(ks[:], ks[:], 1.0, None, Alu.add)

                # threshold: ks-th smallest cost  (c8[:, :4] = -cost ascending cost)
                e4 = pool.tile([M, 4], f32)
                nc.vector.tensor_scalar(e4[:], iota4[0:M, :], ks[:], None, Alu.is_equal)
                tn4 = pool.tile([M, 4], f32)
                nc.vector.tensor_tensor(tn4[:], c8[:, 0:4], e4[:], Alu.mult)
                thn = pool.tile([M, 1], f32)
                nc.vector.tensor_reduce(thn[:], tn4[:], axis=X, op=Alu.add)  # = -thresh
                nc.sync.dma_start(out=scr_thr[:], in_=thn[:])
                ThrN = pool.tile([P, M], f32)
                nc.sync.dma_start(
                    out=ThrN[:], in_=scr_thr[:].flatten().unsqueeze(0).broadcast_to([P, M])
                )
                Thr = pool.tile([P, M], f32)
                nc.vector.tensor_scalar(Thr[:], ThrN[:], -1.0, None, Alu.mult)
                thrb = Thr[:].unsqueeze(1).broadcast_to([P, T, M])

                # ---------------- phase 3: matching + conflict resolution ----------
                match = pool.tile([P, TM], f32)
                nc.vector.tensor_tensor(d3(match[:]), d3(cost[:]), thrb, Alu.is_le)
                nc.vector.tensor_tensor(d3(match[:]), d3(match[:]), srep(5), Alu.mult)
                mgt = pool.tile([P, T], f32)
                nc.vector.tensor_reduce(mgt[:], d3(match[:]), axis=X, op=Alu.add)

                bmib = bmi[:].unsqueeze(1).broadcast_to([P, T, M])
                pm1 = pool.tile([P, TM], f32)
                nc.gpsimd.tensor_tensor(d3(pm1[:]), d3(match[:]), bmib, Alu.mult)
                i1r = pool.tile([P, T], f32)
                nc.vector.tensor_reduce(i1r[:], d3(pm1[:]), axis=X, op=Alu.max)

                nm1 = pool.tile([P, TM], f32)
                nc.vector.tensor_scalar(nm1[:], match[:], -1.0, 1.0, Alu.mult, Alu.add)
                cm = pool.tile([P, TM], f32)
                nc.gpsimd.tensor_tensor(cm[:], cost[:], match[:], Alu.mult)
                nc.vector.scalar_tensor_tensor(cm[:], nm1[:], BIGINF, cm[:], Alu.mult, Alu.add)
                mn2 = pool.tile([P, T], f32)
                nc.vector.tensor_reduce(mn2[:], d3(cm[:]), axis=X, op=Alu.min)
                mn2b = mn2[:].unsqueeze(2).broadcast_to([P, T, M])
                eq2 = pool.tile([P, TM], f32)
                nc.vector.tensor_tensor(d3(eq2[:]), d3(cm[:]), mn2b, Alu.is_equal)
                nc.gpsimd.tensor_tensor(d3(eq2[:]), d3(eq2[:]), bmib, Alu.mult)
                i2r = pool.tile([P, T], f32)
                nc.vector.tensor_reduce(i2r[:], d3(eq2[:]), axis=X, op=Alu.max)

                conf = pool.tile([P, T], f32)
                nc.vector.tensor_scalar(conf[:], mgt[:], 1.0, None, Alu.is_gt)
                asg = pool.tile([P, T], f32)
                nc.vector.tensor_scalar(asg[:], mgt[:], 0.0, None, Alu.is_gt)
                idx1 = pool.tile([P, T], f32)
                nc.vector.tensor_scalar(idx1[:], i1r[:], -1.0, BIG2, Alu.mult, Alu.add)
                idx2 = pool.tile([P, T], f32)
                nc.vector.tensor_scalar(idx2[:], i2r[:], -1.0, BIG2, Alu.mult, Alu.add)
                didx = pool.tile([P, T], f32)
                nc.vector.tensor_tensor(didx[:], idx2[:], idx1[:], Alu.subtract)
                nc.vector.tensor_tensor(didx[:], conf[:], didx[:], Alu.mult)
                mt = pool.tile([P, T], f32)
                nc.vector.tensor_tensor(mt[:], idx1[:], didx[:], Alu.add)
                nc.vector.tensor_tensor(mt[:], mt[:], asg[:], Alu.mult)
                nc.vector.tensor_tensor(mt[:], mt[:], asg[:], Alu.add)
                nc.vector.tensor_scalar(mt[:], mt[:], -1.0, None, Alu.add)

                asg_i = pool.tile([P, T], dt.int32)
                nc.vector.tensor_copy(asg_i[:], asg[:])
                mt_i = pool.tile([P, T], dt.int32)
                nc.vector.tensor_copy(mt_i[:], mt[:])
                nc.sync.dma_start(out=out_assigned[:].rearrange("(t p) -> p t", p=P), in_=asg_i[:])
                nc.sync.dma_start(out=out_matched[:].rearrange("(t p) -> p t", p=P), in_=mt_i[:])


            for _rep in range(reps):
                _body()
    nc.compile()
    return nc


def _get_nc(n=N, reps=1):
    key = (n, reps)
    if key not in _CACHE:
        _CACHE[key] = _build_nc(n, reps)
    return _CACHE[key]


def kernel(preds, targets, masks, img_w=800, img_h=320):
    from concourse.bass_utils import run_bass_kernel_spmd

    nc = _get_nc(N)
    preds = np.ascontiguousarray(preds, dtype=np.float32)
    targets = np.ascontiguousarray(targets, dtype=np.float32)
    maskf = np.ascontiguousarray(masks, dtype=np.float32)
    in_maps = [
        {"preds": preds[b], "targets": targets[b], "maskf": maskf[b]}
        for b in range(B)
    ]
    res = run_bass_kernel_spmd(nc, in_maps, list(range(B))).results
    assigned = np.stack([res[b]["out_assigned"] for b in range(B)]).astype(bool)
    matched = np.stack([res[b]["out_matched"] for b in range(B)]).astype(np.int32)
    return assigned, matched



# revision 5
# speedup vs baseline: 1.2463x; 1.2463x over previous
"""Trainium2 Bass kernel for CLRNet SimOTA-style assignment (B=8, N=4096, M=32, K=72).

Strategy: pure data-parallel over batch - one batch element per NeuronCore.

v2 layout (vs v1):
  * Heavy stage  D[n,m] = sum_k |p - t2|  as 2 passes: subtract (Pool for nP
    tiles / DVE for the rest) + DVE abs-reduce, batched 2 tiles/instruction.
  * C correction via PE matmul (as v1):  D = Draw - predx @ (1-v)^T.
  * Global maxes via gpsimd.partition_all_reduce (no DMA roundtrip).
  * Threshold broadcast via PE transpose + gpsimd.partition_broadcast.
  * No cost masking on the [P,TM] matrix: masked columns get thr = -inf.
  * cm fused to one scalar_tensor_tensor; mask-multiplies dropped.
  * D transposes + Dm assembly happen per 4-tile group inside the loop.
"""

import os
import sys

sys.path.insert(0, "/opt/trn_rl_repo")

import numpy as np

B, N, M, K = 8, 4096, 32, 72
D_FEAT = 78
IMG_W = 800.0
BIG = 100000.0
BIG2 = 100000.0
BIGINF = 1e30
EPS = 1e-12

N_POOL_SUB = 22  # tiles whose subtract runs on GPSIMD (rest on DVE)

_CACHE = {}


def _build_nc(n=N, reps=1):
    import concourse.bass as bass
    import concourse.bacc as bacc
    import concourse.bass_isa as bass_isa
    import concourse.mybir as mybir
    from concourse.tile import TileContext

    Alu = mybir.AluOpType
    ACT = mybir.ActivationFunctionType
    dt = mybir.dt
    X = mybir.AxisListType.X
    XY = mybir.AxisListType.XY

    P = 128
    T = n // P
    TM = T * M
    NG = T // 4  # 4-tile transpose groups

    nc = bacc.Bacc()

    preds = nc.declare_dram_parameter("preds", [n, D_FEAT], dt.float32, isOutput=False)
    targets = nc.declare_dram_parameter("targets", [M, D_FEAT], dt.float32, isOutput=False)
    maskf = nc.declare_dram_parameter("maskf", [M], dt.float32, isOutput=False)
    out_assigned = nc.declare_dram_parameter("out_assigned", [n], dt.int32, isOutput=True)
    out_matched = nc.declare_dram_parameter("out_matched", [n], dt.int32, isOutput=True)

    scr_t2 = nc.dram_tensor("scr_t2", [M, K], dt.float32)
    scr_small = nc.dram_tensor("scr_small", [8, M], dt.float32)

    with TileContext(nc) as tc:
        with (
            tc.tile_pool(name="const", bufs=1) as cpool,
            tc.tile_pool(name="sb", bufs=1) as pool,
            tc.tile_pool(name="diffp", bufs=3) as diffp,
            tc.tile_pool(name="tmp", bufs=6) as tmp,
            tc.tile_pool(name="ps", bufs=2, space="PSUM") as psum,
            tc.tile_pool(name="pscx", bufs=1, space="PSUM") as pscx,
        ):
            f32 = dt.float32

            # ---------------- constants ----------------
            icol = cpool.tile([P, 1], f32)
            nc.gpsimd.iota(icol[:], pattern=[[0, 1]], channel_multiplier=1,
                           allow_small_or_imprecise_dtypes=True)
            irow = cpool.tile([P, P], f32)
            nc.gpsimd.iota(irow[:], pattern=[[1, P]], channel_multiplier=0,
                           allow_small_or_imprecise_dtypes=True)
            ident = cpool.tile([P, P], f32)
            nc.vector.tensor_scalar(ident[:], irow[:], icol[:], None, Alu.is_equal)
            # BMI[p, m] = BIG2 - m
            bmi = cpool.tile([P, M], f32)
            nc.gpsimd.iota(bmi[:], pattern=[[-1, M]], base=int(BIG2),
                           channel_multiplier=0, allow_small_or_imprecise_dtypes=True)
            # iota4[p, j] = j + 1
            iota4 = cpool.tile([P, 4], f32)
            nc.gpsimd.iota(iota4[:], pattern=[[1, 4]], base=1, channel_multiplier=0,
                           allow_small_or_imprecise_dtypes=True)
            epsc = cpool.tile([P, 1], f32)
            nc.gpsimd.memset(epsc[:], EPS)

            def _body():
                # ---------------- input DMAs ----------------
                P_sb = pool.tile([P, T * D_FEAT], f32)
                pview = preds[:].rearrange("(t p) d -> p t d", p=P)
                tch = max(1, T // 4)
                for t0 in range(0, T, tch):
                    t1 = min(T, t0 + tch)
                    nc.sync.dma_start(
                        out=P_sb[:].rearrange("p (t d) -> p t d", d=D_FEAT)[:, t0:t1],
                        in_=pview[:, t0:t1],
                    )
                T_sb = pool.tile([M, D_FEAT], f32)
                nc.sync.dma_start(out=T_sb[:], in_=targets[:])
                Mk = pool.tile([M, 1], f32)
                nc.sync.dma_start(out=Mk[:], in_=maskf[:].unsqueeze(1))

                Pv = P_sb[:].rearrange("p (t d) -> p t d", d=D_FEAT)

                # ---------------- target-side precompute (M partitions) -------
                tdx = T_sb[:, 6:78]
                v0 = pool.tile([M, K], f32)
                nc.vector.tensor_scalar(v0[:], tdx, 0.0, None, Alu.is_ge)
                v1 = pool.tile([M, K], f32)
                nc.vector.tensor_scalar(v1[:], tdx, IMG_W, None, Alu.is_lt)
                vv = pool.tile([M, K], f32)
                nc.vector.tensor_tensor(vv[:], v0[:], v1[:], Alu.mult)
                t2 = pool.tile([M, K], f32)
                nc.vector.tensor_tensor(t2[:], tdx, vv[:], Alu.mult)
                onemv = pool.tile([M, K], f32)
                nc.vector.tensor_scalar(onemv[:], vv[:], -1.0, 1.0, Alu.mult, Alu.add)
                lenr = pool.tile([M, 1], f32)
                nc.vector.tensor_reduce(lenr[:], vv[:], axis=X, op=Alu.add)
                lenc = pool.tile([M, 1], f32)
                nc.vector.tensor_scalar(lenc[:], lenr[:], 1.0, None, Alu.max)
                invlen = pool.tile([M, 1], f32)
                nc.vector.reciprocal(invlen[:], lenc[:])
                a30 = pool.tile([M, 1], f32)
                nc.vector.tensor_scalar(a30[:], lenr[:], 30.0, None, Alu.mult)
                aeps = pool.tile([M, 1], f32)
                nc.vector.tensor_scalar(aeps[:], a30[:], 1e-9, None, Alu.add)

                # t2 -> broadcast to all partitions (DRAM roundtrip)
                nc.sync.dma_start(out=scr_t2[:], in_=t2[:])
                TgtRep = pool.tile([P, M * K], f32)
                nc.sync.dma_start(
                    out=TgtRep[:],
                    in_=scr_t2[:].flatten().unsqueeze(0).broadcast_to([P, M * K]),
                )
                TgtRep4 = TgtRep[:].rearrange("p (u m k) -> p u m k", u=1, k=K)

                # small per-m rows -> SRep [128, (i, m)]
                spack = pool.tile([M, 8], f32)
                nc.vector.tensor_copy(spack[:, 0:1], T_sb[:, 2:3])   # tx
                nc.vector.tensor_copy(spack[:, 1:2], T_sb[:, 3:4])   # ty
                nc.vector.tensor_copy(spack[:, 2:3], T_sb[:, 4:5])   # tth
                nc.vector.tensor_copy(spack[:, 3:4], T_sb[:, 1:2])   # label
                nc.vector.tensor_copy(spack[:, 4:5], invlen[:])
                nc.vector.tensor_copy(spack[:, 5:6], Mk[:])
                nc.vector.tensor_copy(spack[:, 6:7], Mk[:])
                nc.vector.tensor_copy(spack[:, 7:8], Mk[:])
                nc.sync.dma_start(out=scr_small[:].rearrange("i m -> m i"), in_=spack[:])
                SRep = pool.tile([P, 8 * M], f32)
                nc.sync.dma_start(
                    out=SRep[:],
                    in_=scr_small[:].flatten().unsqueeze(0).broadcast_to([P, 8 * M]),
                )

                def srep(i):
                    return SRep[:, i * M:(i + 1) * M].unsqueeze(1).broadcast_to([P, T, M])

                # onemv transposed [K, M] for the C matmul
                onemvT_ps = pscx.tile([K, M], f32, tag="onemvT")
                nc.tensor.transpose(onemvT_ps[:], onemv[:], ident[0:M, 0:M])
                onemvT = pool.tile([K, M], f32)
                nc.scalar.activation(onemvT[:], onemvT_ps[:], ACT.Copy)

                # predxT [72, n] via PE transposes (4 tiles/psum bank, ACT copy)
                predxT = pool.tile([K, n], f32)
                for g in range(NG):
                    tp = psum.tile([K, 4 * P], f32, tag="tp")
                    for j in range(4):
                        t = 4 * g + j
                        nc.tensor.transpose(tp[:, j * P:(j + 1) * P], Pv[:, t, 6:78],
                                            ident[:])
                    nc.scalar.activation(predxT[:, g * 4 * P:(g + 1) * 4 * P], tp[:],
                                         ACT.Copy)

                # C matmuls into PSUM [P, TM]
                Cps = pscx.tile([P, TM], f32, tag="cps")
                for t in range(T):
                    nc.tensor.matmul(
                        Cps[:, t * M:(t + 1) * M],
                        predxT[:, t * P:(t + 1) * P], onemvT[:],
                        start=True, stop=True,
                    )

                # ---------------- heavy loop: |p - t2| reduce ----------------
                Draw = pool.tile([P, TM], f32)
                Dm = pool.tile([P, TM], f32)
                DTn = pool.tile([M, n], f32)
                Dm3 = Dm[:].rearrange("p (t m) -> p t m", m=M)

                # interleave pool/dve-routed pairs so both engines stay fed
                pair_route = []  # True = Pool subtract
                npo = N_POOL_SUB // 2
                nd = T // 2 - npo
                a, b = npo, nd
                for i in range(T // 2):
                    if a > 0 and (b == 0 or a * nd >= b * npo or i % 2 == 0):
                        pair_route.append(True); a -= 1
                    else:
                        pair_route.append(False); b -= 1

                for pi in range(T // 2):
                    t = 2 * pi
                    pb = Pv[:, t:t + 2, 6:78].unsqueeze(2).broadcast_to([P, 2, M, K])
                    diff = diffp.tile([P, 2 * M * K], f32, tag="diff")
                    dv = diff[:].rearrange("p (u m k) -> p u m k", u=2, k=K)
                    tgtb = TgtRep4.broadcast_to([P, 2, M, K])
                    if pair_route[pi]:
                        nc.gpsimd.tensor_tensor(dv, pb, tgtb, Alu.subtract)
                    else:
                        nc.vector.tensor_tensor(dv, pb, tgtb, Alu.subtract)
                    nc.vector.tensor_reduce(
                        Draw[:, t * M:(t + 2) * M].rearrange("p (u m) -> p u m", u=2),
                        dv, axis=X, op=Alu.add, apply_absolute_value=True,
                    )
                    if pi % 2 == 1:
                        # 4-tile group done -> Dm + transposes
                        g = pi // 2
                        sl = slice(g * 4 * M, (g + 1) * 4 * M)
                        nc.vector.tensor_tensor(Dm[:, sl], Draw[:, sl], Cps[:, sl],
                                                Alu.subtract)
                        tpd = psum.tile([M, 4 * P], f32, tag="tpT")
                        for j in range(4):
                            tt = 4 * g + j
                            nc.tensor.transpose(tpd[:, j * P:(j + 1) * P],
                                                Dm3[:, tt, :], ident[:])
                        nc.scalar.activation(DTn[:, g * 4 * P:(g + 1) * 4 * P],
                                             tpd[:], ACT.Copy, scale=-1.0)

                # ---------------- overlapped: xy / th / focal / cls ----------
                def pcol(c):
                    return Pv[:, :, c].unsqueeze(2).broadcast_to([P, T, M])

                d3 = lambda ap: ap.rearrange("p (t m) -> p t m", m=M)

                dxf = tmp.tile([P, TM], f32, tag="tm")
                nc.gpsimd.tensor_tensor(d3(dxf[:]), pcol(2), srep(0), Alu.subtract)
                dyf = tmp.tile([P, TM], f32, tag="tm")
                nc.gpsimd.tensor_tensor(d3(dyf[:]), pcol(3), srep(1), Alu.subtract)
                nc.scalar.activation(dxf[:], dxf[:], ACT.Square)
                nc.scalar.activation(dyf[:], dyf[:], ACT.Square)
                xyf = pool.tile([P, TM], f32)
                nc.vector.tensor_tensor(xyf[:], dxf[:], dyf[:], Alu.add)
                nc.scalar.activation(xyf[:], xyf[:], ACT.Sqrt)
                thf = pool.tile([P, TM], f32)
                nc.gpsimd.tensor_tensor(d3(thf[:]), pcol(4), srep(2), Alu.subtract)
                nc.scalar.activation(thf[:], thf[:], ACT.Abs)

                # focal [P, T*2]
                lg = Pv[:, :, 0:2]
                sig = pool.tile([P, T * 2], f32)
                nc.scalar.activation(sig[:].rearrange("p (t c) -> p t c", c=2), lg,
                                     ACT.Sigmoid)
                qq = pool.tile([P, T * 2], f32)
                nc.vector.tensor_scalar(qq[:], sig[:], -1.0, 1.0, Alu.mult, Alu.add)
                lp = pool.tile([P, T * 2], f32)
                nc.scalar.activation(lp[:], sig[:], ACT.Ln, bias=epsc[:])
                lq = pool.tile([P, T * 2], f32)
                nc.scalar.activation(lq[:], qq[:], ACT.Ln, bias=epsc[:])
                p2 = pool.tile([P, T * 2], f32)
                nc.vector.tensor_tensor(p2[:], sig[:], sig[:], Alu.mult)
                q2 = pool.tile([P, T * 2], f32)
                nc.vector.tensor_tensor(q2[:], qq[:], qq[:], Alu.mult)
                pos = pool.tile([P, T * 2], f32)
                nc.vector.scalar_tensor_tensor(pos[:], lp[:], -0.25, q2[:],
                                               Alu.mult, Alu.mult)
                neg = pool.tile([P, T * 2], f32)
                nc.vector.scalar_tensor_tensor(neg[:], lq[:], -0.75, p2[:],
                                               Alu.mult, Alu.mult)
                fdiff = pool.tile([P, T * 2], f32)
                nc.vector.tensor_tensor(fdiff[:], pos[:], neg[:], Alu.subtract)
                fv = fdiff[:].rearrange("p (t c) -> p t c", c=2)
                d0b = fv[:, :, 0].unsqueeze(2).broadcast_to([P, T, M])
                ddt = pool.tile([P, T], f32)
                nc.vector.tensor_tensor(ddt[:], fv[:, :, 1], fv[:, :, 0], Alu.subtract)
                ddb = ddt[:].unsqueeze(2).broadcast_to([P, T, M])
                cls = pool.tile([P, TM], f32)
                nc.gpsimd.tensor_tensor(d3(cls[:]), srep(3), ddb, Alu.mult)
                nc.gpsimd.tensor_tensor(d3(cls[:]), d3(cls[:]), d0b, Alu.add)

                # global maxes of xy / th early (partition_all_reduce)
                gxy_in = pool.tile([P, 2], f32)
                nc.vector.tensor_reduce(gxy_in[:, 0:1], d3(xyf[:]), axis=XY, op=Alu.max)
                nc.vector.tensor_reduce(gxy_in[:, 1:2], d3(thf[:]), axis=XY, op=Alu.max)
                gxy_out = pool.tile([P, 2], f32)
                nc.gpsimd.partition_all_reduce(gxy_out[:], gxy_in[:], channels=P,
                                               reduce_op=bass_isa.ReduceOp.max)
                gxy2 = pool.tile([P, 2], f32)
                nc.vector.tensor_scalar(gxy2[:], gxy_out[:], 1e-6, None, Alu.max)
                ginv2 = pool.tile([P, 2], f32)
                nc.vector.reciprocal(ginv2[:], gxy2[:])
                nginv2 = pool.tile([P, 2], f32)
                nc.vector.tensor_scalar(nginv2[:], ginv2[:], -1.0, None, Alu.mult)

                xys = tmp.tile([P, TM], f32, tag="tm")
                nc.scalar.activation(xys[:], xyf[:], ACT.Copy, bias=1.01,
                                     scale=nginv2[:, 0:1])
                ths = tmp.tile([P, TM], f32, tag="tm")
                nc.scalar.activation(ths[:], thf[:], ACT.Copy, bias=1.01,
                                     scale=nginv2[:, 1:2])
                Ef = pool.tile([P, TM], f32)
                nc.gpsimd.tensor_tensor(Ef[:], xys[:], ths[:], Alu.mult)

                # d8 from DTn (negated D, descending) -> iou -> ks  (early path)
                d8 = pool.tile([M, 8], f32)
                nc.vector.max(d8[:], DTn[:])
                num4 = pool.tile([M, 4], f32)
                nc.vector.tensor_scalar(num4[:], d8[:, 0:4], a30[:], None, Alu.add)
                den4 = pool.tile([M, 4], f32)
                nc.vector.tensor_scalar(den4[:], d8[:, 0:4], -1.0, aeps[:],
                                        Alu.mult, Alu.add)
                rec4 = pool.tile([M, 4], f32)
                nc.vector.reciprocal(rec4[:], den4[:])
                iou4 = pool.tile([M, 4], f32)
                nc.vector.tensor_tensor(iou4[:], num4[:], rec4[:], Alu.mult)
                nc.vector.tensor_scalar(iou4[:], iou4[:], Mk[:], 0.0, Alu.mult, Alu.max)
                S4 = pool.tile([M, 1], f32)
                nc.vector.tensor_reduce(S4[:], iou4[:], axis=X, op=Alu.add)
                ge2 = pool.tile([M, 1], f32)
                nc.vector.tensor_scalar(ge2[:], S4[:], 2.0, None, Alu.is_ge)
                ge3 = pool.tile([M, 1], f32)
                nc.vector.tensor_scalar(ge3[:], S4[:], 3.0, None, Alu.is_ge)
                ks = pool.tile([M, 1], f32)
                nc.vector.tensor_scalar(ks[:], S4[:], 4.0, None, Alu.is_ge)
                nc.vector.tensor_tensor(ks[:], ks[:], ge2[:], Alu.add)
                nc.vector.tensor_tensor(ks[:], ks[:], ge3[:], Alu.add)
                nc.vector.tensor_scalar(ks[:], ks[:], 1.0, None, Alu.add)
                e4 = pool.tile([M, 4], f32)
                nc.vector.tensor_scalar(e4[:], iota4[0:M, :], ks[:], None, Alu.is_equal)

                # ---------------- tail: dist -> cost ----------------
                dist = tmp.tile([P, TM], f32, tag="tm")
                nc.vector.tensor_tensor(d3(dist[:]), d3(Dm[:]), srep(4), Alu.mult)
                gd_in = pool.tile([P, 1], f32)
                nc.vector.tensor_reduce(gd_in[:], d3(dist[:]), axis=XY, op=Alu.max)
                gd_out = pool.tile([P, 1], f32)
                nc.gpsimd.partition_all_reduce(gd_out[:], gd_in[:], channels=P,
                                               reduce_op=bass_isa.ReduceOp.max)
                gd2 = pool.tile([P, 1], f32)
                nc.vector.tensor_scalar(gd2[:], gd_out[:], 1e-6, None, Alu.max)
                gdi = pool.tile([P, 1], f32)
                nc.vector.reciprocal(gdi[:], gd2[:])
                ngdi = pool.tile([P, 1], f32)
                nc.vector.tensor_scalar(ngdi[:], gdi[:], -1.0, None, Alu.mult)

                ds_ = tmp.tile([P, TM], f32, tag="tm")
                nc.scalar.activation(ds_[:], dist[:], ACT.Copy, bias=1.01,
                                     scale=ngdi[:])
                s3 = tmp.tile([P, TM], f32, tag="tm")
                nc.vector.tensor_tensor(s3[:], ds_[:], Ef[:], Alu.mult)
                sq3 = tmp.tile([P, TM], f32, tag="tm")
                nc.scalar.activation(sq3[:], s3[:], ACT.Square, scale=1.7320508)
                cost = pool.tile([P, TM], f32)
                nc.vector.tensor_tensor(cost[:], cls[:], sq3[:], Alu.subtract)

                # cost transposes (negated)
                costTn = pool.tile([M, n], f32)
                cv = cost[:].rearrange("p (t m) -> p t m", m=M)
                for g in range(NG):
                    tpg = psum.tile([M, 4 * P], f32, tag="tpT")
                    for j in range(4):
                        t = 4 * g + j
                        nc.tensor.transpose(tpg[:, j * P:(j + 1) * P], cv[:, t, :],
                                            ident[:])
                    if g % 2 == 0:
                        nc.scalar.activation(costTn[:, g * 4 * P:(g + 1) * 4 * P],
                                             tpg[:], ACT.Copy, scale=-1.0)
                    else:
                        nc.vector.tensor_scalar(
                            costTn[:, g * 4 * P:(g + 1) * 4 * P], tpg[:],
                            -1.0, None, Alu.mult)

                # threshold: ks-th smallest cost per column
                c8 = pool.tile([M, 8], f32)
                nc.vector.max(c8[:], costTn[:])
                tn4 = pool.tile([M, 4], f32)
                nc.vector.tensor_tensor(tn4[:], c8[:, 0:4], e4[:], Alu.mult)
                thn = pool.tile([M, 1], f32)
                nc.vector.tensor_reduce(thn[:], tn4[:], axis=X, op=Alu.add)  # -thresh
                # mask columns: thn_e = thn*mk + BIGINF*(1-mk), absorption-free
                thn_e = pool.tile([M, 1], f32)
                nc.vector.tensor_scalar(thn_e[:], thn[:], Mk[:], None, Alu.mult)
                tmsk = pool.tile([M, 1], f32)
                nc.vector.tensor_scalar(tmsk[:], Mk[:], -BIGINF, BIGINF,
                                        Alu.mult, Alu.add)
                nc.vector.tensor_tensor(thn_e[:], thn_e[:], tmsk[:], Alu.add)
                # transpose [M,1] -> [1,M], negate, broadcast to all partitions
                thT_ps = pscx.tile([1, M], f32, tag="thT")
                nc.tensor.transpose(thT_ps[:], thn_e[:], ident[0:M, 0:M])
                thT = pool.tile([1, M], f32)
                nc.vector.tensor_scalar(thT[:], thT_ps[:], -1.0, None, Alu.mult)
                Thr = pool.tile([P, M], f32)
                nc.gpsimd.partition_broadcast(Thr[:], thT[:], channels=P)
                thrb = Thr[:].unsqueeze(1).broadcast_to([P, T, M])

                # ---------------- matching + conflict resolution ----------
                match = pool.tile([P, TM], f32)
                nc.vector.tensor_tensor(d3(match[:]), d3(cost[:]), thrb, Alu.is_le)
                mgt = pool.tile([P, T], f32)
                nc.vector.tensor_reduce(mgt[:], d3(match[:]), axis=X, op=Alu.add)

                bmib = bmi[:].unsqueeze(1).broadcast_to([P, T, M])
                pm1 = tmp.tile([P, TM], f32, tag="tm")
                nc.gpsimd.tensor_tensor(d3(pm1[:]), d3(match[:]), bmib, Alu.mult)
                i1r = pool.tile([P, T], f32)
                nc.vector.tensor_reduce(i1r[:], d3(pm1[:]), axis=X, op=Alu.max)

                q1 = tmp.tile([P, TM], f32, tag="tm")
                nc.vector.tensor_scalar(q1[:], match[:], -BIGINF, BIGINF,
                                        Alu.mult, Alu.add)
                cm = tmp.tile([P, TM], f32, tag="tm")
                nc.vector.tensor_tensor(cm[:], cost[:], q1[:], Alu.add)
                mn2 = pool.tile([P, T], f32)
                nc.vector.tensor_reduce(mn2[:], d3(cm[:]), axis=X, op=Alu.min)
                mn2b = mn2[:].unsqueeze(2).broadcast_to([P, T, M])
                eq2 = tmp.tile([P, TM], f32, tag="tm")
                nc.vector.tensor_tensor(d3(eq2[:]), d3(cm[:]), mn2b, Alu.is_equal)
                nc.gpsimd.tensor_tensor(d3(eq2[:]), d3(eq2[:]), bmib, Alu.mult)
                i2r = pool.tile([P, T], f32)
                nc.vector.tensor_reduce(i2r[:], d3(eq2[:]), axis=X, op=Alu.max)

                conf = pool.tile([P, T], f32)
                nc.vector.tensor_scalar(conf[:], mgt[:], 1.0, None, Alu.is_gt)
                asg = pool.tile([P, T], f32)
                nc.vector.tensor_scalar(asg[:], mgt[:], 0.0, None, Alu.is_gt)
                idx1 = pool.tile([P, T], f32)
                nc.vector.tensor_scalar(idx1[:], i1r[:], -1.0, BIG2, Alu.mult, Alu.add)
                idx2 = pool.tile([P, T], f32)
                nc.vector.tensor_scalar(idx2[:], i2r[:], -1.0, BIG2, Alu.mult, Alu.add)
                didx = pool.tile([P, T], f32)
                nc.vector.tensor_tensor(didx[:], idx2[:], idx1[:], Alu.subtract)
                nc.vector.tensor_tensor(didx[:], conf[:], didx[:], Alu.mult)
                mt = pool.tile([P, T], f32)
                nc.vector.tensor_tensor(mt[:], idx1[:], didx[:], Alu.add)
                nc.vector.tensor_tensor(mt[:], mt[:], asg[:], Alu.mult)
                nc.vector.tensor_tensor(mt[:], mt[:], asg[:], Alu.add)
                nc.vector.tensor_scalar(mt[:], mt[:], -1.0, None, Alu.add)

                asg_i = pool.tile([P, T], dt.int32)
                nc.vector.tensor_copy(asg_i[:], asg[:])
                mt_i = pool.tile([P, T], dt.int32)
                nc.vector.tensor_copy(mt_i[:], mt[:])
                nc.sync.dma_start(out=out_assigned[:].rearrange("(t p) -> p t", p=P),
                                  in_=asg_i[:])
                nc.sync.dma_start(out=out_matched[:].rearrange("(t p) -> p t", p=P),
                                  in_=mt_i[:])

            for _rep in range(reps):
                _body()
    nc.compile()
    return nc


def _get_nc(n=N, reps=1):
    key = (n, reps)
    if key not in _CACHE:
        _CACHE[key] = _build_nc(n, reps)
    return _CACHE[key]


def kernel(preds, targets, masks, img_w=800, img_h=320):
    from concourse.bass_utils import run_bass_kernel_spmd

    nc = _get_nc(N)
    preds = np.ascontiguousarray(preds, dtype=np.float32)
    targets = np.ascontiguousarray(targets, dtype=np.float32)
    maskf = np.ascontiguousarray(masks, dtype=np.float32)
    in_maps = [
        {"preds": preds[b], "targets": targets[b], "maskf": maskf[b]}
        for b in range(B)
    ]
    res = run_bass_kernel_spmd(nc, in_maps, list(range(B))).results
    assigned = np.stack([res[b]["out_assigned"] for b in range(B)]).astype(bool)
    matched = np.stack([res[b]["out_matched"] for b in range(B)]).astype(np.int32)
    return assigned, matched


# revision 6
# speedup vs baseline: 1.2972x; 1.0408x over previous
"""Trainium2 Bass kernel for CLRNet SimOTA-style assignment (B=8, N=4096, M=32, K=72).

Strategy: pure data-parallel over batch - one batch element per NeuronCore.

v4:
  * p-major n layout: partition p holds rows n = p*32 + t (contiguous DMA
    descriptors for preds load and output store).
  * Heavy stage: Pool subtract (2-tile batched) / DVE subtract (per tile),
    DVE abs-reduce (2-tile batched).  D = Draw - predx @ (1-v)^T (PE).
  * Small/medium work interleaved into the heavy loop's emission order.
  * Global maxes via gpsimd.partition_all_reduce; threshold broadcast via
    PE transpose + partition_broadcast; latency-critical tail on DVE only.
"""

import os
import sys

sys.path.insert(0, "/opt/trn_rl_repo")

import numpy as np

B, N, M, K = 8, 4096, 32, 72
D_FEAT = 78
IMG_W = 800.0
BIG = 100000.0
BIG2 = 100000.0
BIGINF = 1e30
EPS = 1e-12

N_POOL_SUB = 22  # tiles whose subtract runs on GPSIMD (rest on DVE)

_CACHE = {}


def _build_nc(n=N, reps=1):
    import concourse.bass as bass
    import concourse.bacc as bacc
    import concourse.bass_isa as bass_isa
    import concourse.mybir as mybir
    from concourse.tile import TileContext

    Alu = mybir.AluOpType
    ACT = mybir.ActivationFunctionType
    dt = mybir.dt
    X = mybir.AxisListType.X
    XY = mybir.AxisListType.XY

    P = 128
    T = n // P
    TM = T * M
    NG = T // 4

    nc = bacc.Bacc()

    preds = nc.declare_dram_parameter("preds", [n, D_FEAT], dt.float32, isOutput=False)
    targets = nc.declare_dram_parameter("targets", [M, D_FEAT], dt.float32, isOutput=False)
    maskf = nc.declare_dram_parameter("maskf", [M], dt.float32, isOutput=False)
    out_assigned = nc.declare_dram_parameter("out_assigned", [n], dt.int32, isOutput=True)
    out_matched = nc.declare_dram_parameter("out_matched", [n], dt.int32, isOutput=True)

    scr_t2 = nc.dram_tensor("scr_t2", [M, K], dt.float32)
    scr_small = nc.dram_tensor("scr_small", [8, M], dt.float32)

    with TileContext(nc) as tc:
        with (
            tc.tile_pool(name="const", bufs=1) as cpool,
            tc.tile_pool(name="sb", bufs=1) as pool,
            tc.tile_pool(name="diffp", bufs=3) as diffp,
            tc.tile_pool(name="tmp", bufs=6) as tmp,
            tc.tile_pool(name="ps", bufs=2, space="PSUM") as psum,
            tc.tile_pool(name="pscx", bufs=1, space="PSUM") as pscx,
        ):
            f32 = dt.float32

            # ---------------- constants ----------------
            icol = cpool.tile([P, 1], f32)
            nc.gpsimd.iota(icol[:], pattern=[[0, 1]], channel_multiplier=1,
                           allow_small_or_imprecise_dtypes=True)
            irow = cpool.tile([P, P], f32)
            nc.gpsimd.iota(irow[:], pattern=[[1, P]], channel_multiplier=0,
                           allow_small_or_imprecise_dtypes=True)
            ident = cpool.tile([P, P], f32)
            nc.vector.tensor_scalar(ident[:], irow[:], icol[:], None, Alu.is_equal)
            bmi = cpool.tile([P, M], f32)
            nc.gpsimd.iota(bmi[:], pattern=[[-1, M]], base=int(BIG2),
                           channel_multiplier=0, allow_small_or_imprecise_dtypes=True)
            iota4 = cpool.tile([P, 4], f32)
            nc.gpsimd.iota(iota4[:], pattern=[[1, 4]], base=1, channel_multiplier=0,
                           allow_small_or_imprecise_dtypes=True)
            epsc = cpool.tile([P, 1], f32)
            nc.gpsimd.memset(epsc[:], EPS)

            def _body():
                # ---------------- input DMAs (targets first) ----------------
                T_sb = pool.tile([M, D_FEAT], f32)
                nc.sync.dma_start(out=T_sb[:], in_=targets[:])
                Mk = pool.tile([M, 1], f32)
                nc.sync.dma_start(out=Mk[:], in_=maskf[:].unsqueeze(1))

                # p-major: partition p holds rows n = p*T + t  (contiguous)
                P_sb = pool.tile([P, T * D_FEAT], f32)
                pview = preds[:].rearrange("(p t) d -> p t d", p=P)
                tch = max(1, T // 4)
                for t0 in range(0, T, tch):
                    t1 = min(T, t0 + tch)
                    nc.sync.dma_start(
                        out=P_sb[:].rearrange("p (t d) -> p t d", d=D_FEAT)[:, t0:t1],
                        in_=pview[:, t0:t1],
                    )
                Pv = P_sb[:].rearrange("p (t d) -> p t d", d=D_FEAT)

                # ---------------- target-side precompute (M partitions) -------
                tdx = T_sb[:, 6:78]
                v0 = pool.tile([M, K], f32)
                nc.vector.tensor_scalar(v0[:], tdx, 0.0, None, Alu.is_ge)
                v1 = pool.tile([M, K], f32)
                nc.vector.tensor_scalar(v1[:], tdx, IMG_W, None, Alu.is_lt)
                vv = pool.tile([M, K], f32)
                nc.vector.tensor_tensor(vv[:], v0[:], v1[:], Alu.mult)
                t2 = pool.tile([M, K], f32)
                nc.vector.tensor_tensor(t2[:], tdx, vv[:], Alu.mult)
                nc.sync.dma_start(out=scr_t2[:], in_=t2[:])
                TgtRep = pool.tile([P, M * K], f32)
                nc.sync.dma_start(
                    out=TgtRep[:],
                    in_=scr_t2[:].flatten().unsqueeze(0).broadcast_to([P, M * K]),
                )
                TgtRep3 = TgtRep[:].rearrange("p (m k) -> p m k", k=K)
                TgtRep4 = TgtRep[:].rearrange("p (u m k) -> p u m k", u=1, k=K)

                onemv = pool.tile([M, K], f32)
                nc.vector.tensor_scalar(onemv[:], vv[:], -1.0, 1.0, Alu.mult, Alu.add)
                lenr = pool.tile([M, 1], f32)
                nc.vector.tensor_reduce(lenr[:], vv[:], axis=X, op=Alu.add)
                lenc = pool.tile([M, 1], f32)
                nc.vector.tensor_scalar(lenc[:], lenr[:], 1.0, None, Alu.max)
                invlen = pool.tile([M, 1], f32)
                nc.vector.reciprocal(invlen[:], lenc[:])
                a30 = pool.tile([M, 1], f32)
                nc.vector.tensor_scalar(a30[:], lenr[:], 30.0, None, Alu.mult)
                aeps = pool.tile([M, 1], f32)
                nc.vector.tensor_scalar(aeps[:], a30[:], 1e-9, None, Alu.add)

                spack = pool.tile([M, 8], f32)
                nc.vector.tensor_copy(spack[:, 0:1], T_sb[:, 2:3])   # tx
                nc.vector.tensor_copy(spack[:, 1:2], T_sb[:, 3:4])   # ty
                nc.vector.tensor_copy(spack[:, 2:3], T_sb[:, 4:5])   # tth
                nc.vector.tensor_copy(spack[:, 3:4], T_sb[:, 1:2])   # label
                nc.vector.tensor_copy(spack[:, 4:5], invlen[:])
                nc.vector.tensor_copy(spack[:, 5:6], Mk[:])
                nc.vector.tensor_copy(spack[:, 6:7], Mk[:])
                nc.vector.tensor_copy(spack[:, 7:8], Mk[:])
                nc.sync.dma_start(out=scr_small[:].rearrange("i m -> m i"), in_=spack[:])
                SRep = pool.tile([P, 8 * M], f32)
                nc.sync.dma_start(
                    out=SRep[:],
                    in_=scr_small[:].flatten().unsqueeze(0).broadcast_to([P, 8 * M]),
                )

                def srep(i):
                    return SRep[:, i * M:(i + 1) * M].unsqueeze(1).broadcast_to([P, T, M])

                onemvT_ps = pscx.tile([K, M], f32, tag="onemvT")
                nc.tensor.transpose(onemvT_ps[:], onemv[:], ident[0:M, 0:M])
                onemvT = pool.tile([K, M], f32)
                nc.scalar.activation(onemvT[:], onemvT_ps[:], ACT.Copy)

                # predxT [72, n] via PE transposes
                predxT = pool.tile([K, n], f32)
                for g in range(NG):
                    tp = psum.tile([K, 4 * P], f32, tag="tp")
                    for j in range(4):
                        t = 4 * g + j
                        nc.tensor.transpose(tp[:, j * P:(j + 1) * P], Pv[:, t, 6:78],
                                            ident[:])
                    nc.scalar.activation(predxT[:, g * 4 * P:(g + 1) * 4 * P], tp[:],
                                         ACT.Copy)

                Cps = pscx.tile([P, TM], f32, tag="cps")
                for t in range(T):
                    nc.tensor.matmul(
                        Cps[:, t * M:(t + 1) * M],
                        predxT[:, t * P:(t + 1) * P], onemvT[:],
                        start=True, stop=True,
                    )

                # ---------------- persistent [P, TM] tiles ----------------
                Draw = pool.tile([P, TM], f32)
                Dm = pool.tile([P, TM], f32)
                DTn = pool.tile([M, n], f32)
                xyf = pool.tile([P, TM], f32)
                thf = pool.tile([P, TM], f32)
                cls = pool.tile([P, TM], f32)
                Ef = pool.tile([P, TM], f32)
                cost = pool.tile([P, TM], f32)
                match = pool.tile([P, TM], f32)
                dist = pool.tile([P, TM], f32)
                Dm3 = Dm[:].rearrange("p (t m) -> p t m", m=M)
                d3 = lambda ap: ap.rearrange("p (t m) -> p t m", m=M)

                def pcol(c):
                    return Pv[:, :, c].unsqueeze(2).broadcast_to([P, T, M])

                # ---------------- filler emission (interleaved) -----------
                dxf = tmp.tile([P, TM], f32, tag="tm")
                dyf = tmp.tile([P, TM], f32, tag="tm")
                gxy_in = pool.tile([P, 2], f32)
                gxy_out = pool.tile([P, 2], f32)
                nginv2 = pool.tile([P, 2], f32)
                xys = tmp.tile([P, TM], f32, tag="tm")
                ths = tmp.tile([P, TM], f32, tag="tm")
                sig = pool.tile([P, T * 2], f32)
                qq = pool.tile([P, T * 2], f32)
                lp = pool.tile([P, T * 2], f32)
                lq = pool.tile([P, T * 2], f32)
                p2 = pool.tile([P, T * 2], f32)
                q2 = pool.tile([P, T * 2], f32)
                pos = pool.tile([P, T * 2], f32)
                neg = pool.tile([P, T * 2], f32)
                fdiff = pool.tile([P, T * 2], f32)
                ddt = pool.tile([P, T], f32)
                gd_in = pool.tile([P, 1], f32)

                def f_pool_subs():
                    nc.gpsimd.tensor_tensor(d3(dxf[:]), pcol(2), srep(0), Alu.subtract)
                    nc.gpsimd.tensor_tensor(d3(dyf[:]), pcol(3), srep(1), Alu.subtract)
                    nc.gpsimd.tensor_tensor(d3(thf[:]), pcol(4), srep(2), Alu.subtract)

                def f_squares():
                    nc.scalar.activation(dxf[:], dxf[:], ACT.Square)
                    nc.scalar.activation(dyf[:], dyf[:], ACT.Square)
                    nc.scalar.activation(thf[:], thf[:], ACT.Abs)

                def f_xyf():
                    nc.vector.tensor_tensor(xyf[:], dxf[:], dyf[:], Alu.add)
                    nc.scalar.activation(xyf[:], xyf[:], ACT.Sqrt)

                def f_focal1():
                    lg = Pv[:, :, 0:2]
                    nc.scalar.activation(sig[:].rearrange("p (t c) -> p t c", c=2),
                                         lg, ACT.Sigmoid)
                    nc.vector.tensor_scalar(qq[:], sig[:], -1.0, 1.0, Alu.mult, Alu.add)
                    nc.scalar.activation(lp[:], sig[:], ACT.Ln, bias=epsc[:])
                    nc.scalar.activation(lq[:], qq[:], ACT.Ln, bias=epsc[:])

                def f_focal2():
                    nc.vector.tensor_tensor(p2[:], sig[:], sig[:], Alu.mult)
                    nc.vector.tensor_tensor(q2[:], qq[:], qq[:], Alu.mult)
                    nc.vector.scalar_tensor_tensor(pos[:], lp[:], -0.25, q2[:],
                                                   Alu.mult, Alu.mult)
                    nc.vector.scalar_tensor_tensor(neg[:], lq[:], -0.75, p2[:],
                                                   Alu.mult, Alu.mult)
                    nc.vector.tensor_tensor(fdiff[:], pos[:], neg[:], Alu.subtract)
                    fv = fdiff[:].rearrange("p (t c) -> p t c", c=2)
                    nc.vector.tensor_tensor(ddt[:], fv[:, :, 1], fv[:, :, 0],
                                            Alu.subtract)

                def f_cls():
                    fv = fdiff[:].rearrange("p (t c) -> p t c", c=2)
                    d0b = fv[:, :, 0].unsqueeze(2).broadcast_to([P, T, M])
                    ddb = ddt[:].unsqueeze(2).broadcast_to([P, T, M])
                    nc.gpsimd.tensor_tensor(d3(cls[:]), srep(3), ddb, Alu.mult)
                    nc.gpsimd.tensor_tensor(d3(cls[:]), d3(cls[:]), d0b, Alu.add)

                def f_gxy():
                    nc.vector.tensor_reduce(gxy_in[:, 0:1], d3(xyf[:]), axis=XY,
                                            op=Alu.max)
                    nc.vector.tensor_reduce(gxy_in[:, 1:2], d3(thf[:]), axis=XY,
                                            op=Alu.max)
                    nc.gpsimd.partition_all_reduce(gxy_out[:], gxy_in[:], channels=P,
                                                   reduce_op=bass_isa.ReduceOp.max)

                def f_nginv():
                    g2 = pool.tile([P, 2], f32)
                    nc.vector.tensor_scalar(g2[:], gxy_out[:], 1e-6, None, Alu.max)
                    gi = pool.tile([P, 2], f32)
                    nc.vector.reciprocal(gi[:], g2[:])
                    nc.vector.tensor_scalar(nginv2[:], gi[:], -1.0, None, Alu.mult)
                    nc.scalar.activation(xys[:], xyf[:], ACT.Copy, bias=1.01,
                                         scale=nginv2[:, 0:1])
                    nc.scalar.activation(ths[:], thf[:], ACT.Copy, bias=1.01,
                                         scale=nginv2[:, 1:2])

                def f_ef():
                    nc.gpsimd.tensor_tensor(Ef[:], xys[:], ths[:], Alu.mult)

                fillers = [f_pool_subs, f_squares, f_xyf, f_focal1, f_focal2,
                           f_cls, f_gxy, f_nginv, f_ef]

                # ---------------- heavy loop ----------------
                pair_route = []
                npo = N_POOL_SUB // 2
                nd = T // 2 - npo
                a, b = npo, nd
                for i in range(T // 2):
                    if a > 0 and (b == 0 or a * nd >= b * npo or i % 2 == 0):
                        pair_route.append(True); a -= 1
                    else:
                        pair_route.append(False); b -= 1

                fi = 0
                for pi in range(T // 2):
                    t = 2 * pi
                    diff = diffp.tile([P, 2 * M * K], f32, tag="diff")
                    dv = diff[:].rearrange("p (u m k) -> p u m k", u=2, k=K)
                    if pair_route[pi]:
                        pb = Pv[:, t:t + 2, 6:78].unsqueeze(2).broadcast_to([P, 2, M, K])
                        tgtb = TgtRep4.broadcast_to([P, 2, M, K])
                        nc.gpsimd.tensor_tensor(dv, pb, tgtb, Alu.subtract)
                    else:
                        for u in range(2):
                            pbu = Pv[:, t + u, 6:78].unsqueeze(1).broadcast_to([P, M, K])
                            nc.vector.tensor_tensor(dv[:, u], pbu, TgtRep3,
                                                    Alu.subtract)
                    nc.vector.tensor_reduce(
                        Draw[:, t * M:(t + 2) * M].rearrange("p (u m) -> p u m", u=2),
                        dv, axis=X, op=Alu.add, apply_absolute_value=True,
                    )
                    if pi % 2 == 1:
                        g = pi // 2
                        sl = slice(g * 4 * M, (g + 1) * 4 * M)
                        nc.vector.tensor_tensor(Dm[:, sl], Draw[:, sl], Cps[:, sl],
                                                Alu.subtract)
                        # dist for this group + running per-partition max
                        nc.vector.tensor_tensor(
                            d3(dist[:])[:, 4 * g:4 * g + 4, :],
                            Dm3[:, 4 * g:4 * g + 4, :], srep(4)[:, 0:4, :], Alu.mult)
                        if g == 0:
                            nc.vector.tensor_reduce(
                                gd_in[:],
                                dist[:, sl].rearrange("p (a q) -> p a q", a=1),
                                axis=XY, op=Alu.max)
                        else:
                            gdp = pool.tile([P, 1], f32, tag="gdp")
                            nc.vector.tensor_reduce(
                                gdp[:],
                                dist[:, sl].rearrange("p (a q) -> p a q", a=1),
                                axis=XY, op=Alu.max)
                            nc.vector.tensor_tensor(gd_in[:], gd_in[:], gdp[:],
                                                    Alu.max)
                        tpd = psum.tile([M, 4 * P], f32, tag="tpT")
                        for j in range(4):
                            tt = 4 * g + j
                            nc.tensor.transpose(tpd[:, j * P:(j + 1) * P],
                                                Dm3[:, tt, :], ident[:])
                        nc.scalar.activation(DTn[:, g * 4 * P:(g + 1) * 4 * P],
                                             tpd[:], ACT.Copy, scale=-1.0)
                    if fi < len(fillers):
                        fillers[fi](); fi += 1
                while fi < len(fillers):
                    fillers[fi](); fi += 1

                # ---------------- d8 / iou / ks ----------------
                d8 = pool.tile([M, 8], f32)
                nc.vector.max(d8[:], DTn[:])
                num4 = pool.tile([M, 4], f32)
                nc.vector.tensor_scalar(num4[:], d8[:, 0:4], a30[:], None, Alu.add)
                den4 = pool.tile([M, 4], f32)
                nc.vector.tensor_scalar(den4[:], d8[:, 0:4], -1.0, aeps[:],
                                        Alu.mult, Alu.add)
                rec4 = pool.tile([M, 4], f32)
                nc.vector.reciprocal(rec4[:], den4[:])
                iou4 = pool.tile([M, 4], f32)
                nc.vector.tensor_tensor(iou4[:], num4[:], rec4[:], Alu.mult)
                nc.vector.tensor_scalar(iou4[:], iou4[:], Mk[:], 0.0, Alu.mult, Alu.max)
                S4 = pool.tile([M, 1], f32)
                nc.vector.tensor_reduce(S4[:], iou4[:], axis=X, op=Alu.add)
                ge2 = pool.tile([M, 1], f32)
                nc.vector.tensor_scalar(ge2[:], S4[:], 2.0, None, Alu.is_ge)
                ge3 = pool.tile([M, 1], f32)
                nc.vector.tensor_scalar(ge3[:], S4[:], 3.0, None, Alu.is_ge)
                ks = pool.tile([M, 1], f32)
                nc.vector.tensor_scalar(ks[:], S4[:], 4.0, None, Alu.is_ge)
                nc.vector.tensor_tensor(ks[:], ks[:], ge2[:], Alu.add)
                nc.vector.tensor_tensor(ks[:], ks[:], ge3[:], Alu.add)
                nc.vector.tensor_scalar(ks[:], ks[:], 1.0, None, Alu.add)
                e4 = pool.tile([M, 4], f32)
                nc.vector.tensor_scalar(e4[:], iota4[0:M, :], ks[:], None, Alu.is_equal)

                # ---------------- tail: cost ----------------
                gd_out = pool.tile([P, 1], f32)
                nc.gpsimd.partition_all_reduce(gd_out[:], gd_in[:], channels=P,
                                               reduce_op=bass_isa.ReduceOp.max)
                gd2 = pool.tile([P, 1], f32)
                nc.vector.tensor_scalar(gd2[:], gd_out[:], 1e-6, None, Alu.max)
                gdi = pool.tile([P, 1], f32)
                nc.vector.reciprocal(gdi[:], gd2[:])
                ngdi = pool.tile([P, 1], f32)
                nc.vector.tensor_scalar(ngdi[:], gdi[:], -1.0, None, Alu.mult)

                ds_ = tmp.tile([P, TM], f32, tag="tm")
                nc.scalar.activation(ds_[:], dist[:], ACT.Copy, bias=1.01,
                                     scale=ngdi[:])
                s3 = tmp.tile([P, TM], f32, tag="tm")
                nc.vector.tensor_tensor(s3[:], ds_[:], Ef[:], Alu.mult)
                sq3 = tmp.tile([P, TM], f32, tag="tm")
                nc.scalar.activation(sq3[:], s3[:], ACT.Square, scale=1.7320508)
                nc.vector.tensor_tensor(cost[:], cls[:], sq3[:], Alu.subtract)

                costTn = pool.tile([M, n], f32)
                cv = cost[:].rearrange("p (t m) -> p t m", m=M)
                for g in range(NG):
                    tpg = psum.tile([M, 4 * P], f32, tag="tpT")
                    for j in range(4):
                        t = 4 * g + j
                        nc.tensor.transpose(tpg[:, j * P:(j + 1) * P], cv[:, t, :],
                                            ident[:])
                    if g % 2 == 0:
                        nc.scalar.activation(costTn[:, g * 4 * P:(g + 1) * 4 * P],
                                             tpg[:], ACT.Copy, scale=-1.0)
                    else:
                        nc.vector.tensor_scalar(
                            costTn[:, g * 4 * P:(g + 1) * 4 * P], tpg[:],
                            -1.0, None, Alu.mult)

                c8 = pool.tile([M, 8], f32)
                nc.vector.max(c8[:], costTn[:])
                tn4 = pool.tile([M, 4], f32)
                nc.vector.tensor_tensor(tn4[:], c8[:, 0:4], e4[:], Alu.mult)
                thn = pool.tile([M, 1], f32)
                nc.vector.tensor_reduce(thn[:], tn4[:], axis=X, op=Alu.add)  # -thresh
                thn_e = pool.tile([M, 1], f32)
                nc.vector.tensor_scalar(thn_e[:], thn[:], Mk[:], None, Alu.mult)
                tmsk = pool.tile([M, 1], f32)
                nc.vector.tensor_scalar(tmsk[:], Mk[:], -BIGINF, BIGINF,
                                        Alu.mult, Alu.add)
                nc.vector.tensor_tensor(thn_e[:], thn_e[:], tmsk[:], Alu.add)
                thT_ps = pscx.tile([1, M], f32, tag="thT")
                nc.tensor.transpose(thT_ps[:], thn_e[:], ident[0:M, 0:M])
                thT = pool.tile([1, M], f32)
                nc.vector.tensor_scalar(thT[:], thT_ps[:], -1.0, None, Alu.mult)
                Thr = pool.tile([P, M], f32)
                nc.gpsimd.partition_broadcast(Thr[:], thT[:], channels=P)
                thrb = Thr[:].unsqueeze(1).broadcast_to([P, T, M])

                # ---------------- matching + conflict resolution ----------
                nc.vector.tensor_tensor(d3(match[:]), d3(cost[:]), thrb, Alu.is_le)
                mgt = pool.tile([P, T], f32)
                nc.vector.tensor_reduce(mgt[:], d3(match[:]), axis=X, op=Alu.add)

                bmib = bmi[:].unsqueeze(1).broadcast_to([P, T, M])
                pm1 = tmp.tile([P, TM], f32, tag="tm")
                nc.vector.scalar_tensor_tensor(d3(pm1[:]), d3(match[:]), 1.0, bmib,
                                               Alu.mult, Alu.mult)
                i1r = pool.tile([P, T], f32)
                nc.vector.tensor_reduce(i1r[:], d3(pm1[:]), axis=X, op=Alu.max)

                q1 = tmp.tile([P, TM], f32, tag="tm")
                nc.vector.tensor_scalar(q1[:], match[:], -BIGINF, BIGINF,
                                        Alu.mult, Alu.add)
                cm = tmp.tile([P, TM], f32, tag="tm")
                nc.vector.tensor_tensor(cm[:], cost[:], q1[:], Alu.add)
                mn2 = pool.tile([P, T], f32)
                nc.vector.tensor_reduce(mn2[:], d3(cm[:]), axis=X, op=Alu.min)
                mn2b = mn2[:].unsqueeze(2).broadcast_to([P, T, M])
                eq2 = tmp.tile([P, TM], f32, tag="tm")
                nc.vector.tensor_tensor(d3(eq2[:]), d3(cm[:]), mn2b, Alu.is_equal)
                eq2b = tmp.tile([P, TM], f32, tag="tm")
                nc.vector.scalar_tensor_tensor(d3(eq2b[:]), d3(eq2[:]), 1.0, bmib,
                                               Alu.mult, Alu.mult)
                i2r = pool.tile([P, T], f32)
                nc.vector.tensor_reduce(i2r[:], d3(eq2b[:]), axis=X, op=Alu.max)

                conf = pool.tile([P, T], f32)
                nc.vector.tensor_scalar(conf[:], mgt[:], 1.0, None, Alu.is_gt)
                asg = pool.tile([P, T], f32)
                nc.vector.tensor_scalar(asg[:], mgt[:], 0.0, None, Alu.is_gt)
                idx1 = pool.tile([P, T], f32)
                nc.vector.tensor_scalar(idx1[:], i1r[:], -1.0, BIG2, Alu.mult, Alu.add)
                idx2 = pool.tile([P, T], f32)
                nc.vector.tensor_scalar(idx2[:], i2r[:], -1.0, BIG2, Alu.mult, Alu.add)
                didx = pool.tile([P, T], f32)
                nc.vector.tensor_tensor(didx[:], idx2[:], idx1[:], Alu.subtract)
                nc.vector.tensor_tensor(didx[:], conf[:], didx[:], Alu.mult)
                mt = pool.tile([P, T], f32)
                nc.vector.tensor_tensor(mt[:], idx1[:], didx[:], Alu.add)
                nc.vector.tensor_tensor(mt[:], mt[:], asg[:], Alu.mult)
                nc.vector.tensor_tensor(mt[:], mt[:], asg[:], Alu.add)
                nc.vector.tensor_scalar(mt[:], mt[:], -1.0, None, Alu.add)

                asg_i = pool.tile([P, T], dt.int32)
                nc.vector.tensor_copy(asg_i[:], asg[:])
                mt_i = pool.tile([P, T], dt.int32)
                nc.vector.tensor_copy(mt_i[:], mt[:])
                # p-major: DRAM n = p*T + t -> per-partition contiguous runs
                nc.sync.dma_start(out=out_assigned[:].rearrange("(p t) -> p t", p=P),
                                  in_=asg_i[:])
                nc.sync.dma_start(out=out_matched[:].rearrange("(p t) -> p t", p=P),
                                  in_=mt_i[:])

            for _rep in range(reps):
                _body()
    nc.compile()
    return nc


def _get_nc(n=N, reps=1):
    key = (n, reps)
    if key not in _CACHE:
        _CACHE[key] = _build_nc(n, reps)
    return _CACHE[key]


def kernel(preds, targets, masks, img_w=800, img_h=320):
    from concourse.bass_utils import run_bass_kernel_spmd

    nc = _get_nc(N)
    preds = np.ascontiguousarray(preds, dtype=np.float32)
    targets = np.ascontiguousarray(targets, dtype=np.float32)
    maskf = np.ascontiguousarray(masks, dtype=np.float32)
    in_maps = [
        {"preds": preds[b], "targets": targets[b], "maskf": maskf[b]}
        for b in range(B)
    ]
    res = run_bass_kernel_spmd(nc, in_maps, list(range(B))).results
    assigned = np.stack([res[b]["out_assigned"] for b in range(B)]).astype(bool)
    matched = np.stack([res[b]["out_matched"] for b in range(B)]).astype(np.int32)
    return assigned, matched


# revision 7
# speedup vs baseline: 1.2980x; 1.0006x over previous
"""Trainium2 Bass kernel for CLRNet SimOTA-style assignment (B=8, N=4096, M=32, K=72).

Strategy: pure data-parallel over batch - one batch element per NeuronCore.

v4:
  * p-major n layout: partition p holds rows n = p*32 + t (contiguous DMA
    descriptors for preds load and output store).
  * Heavy stage: Pool subtract (2-tile batched) / DVE subtract (per tile),
    DVE abs-reduce (2-tile batched).  D = Draw - predx @ (1-v)^T (PE).
  * Small/medium work interleaved into the heavy loop's emission order.
  * Global maxes via gpsimd.partition_all_reduce; threshold broadcast via
    PE transpose + partition_broadcast; latency-critical tail on DVE only.
"""

import os
import sys

sys.path.insert(0, "/opt/trn_rl_repo")

import numpy as np

B, N, M, K = 8, 4096, 32, 72
D_FEAT = 78
IMG_W = 800.0
BIG = 100000.0
BIG2 = 100000.0
BIGINF = 1e30
EPS = 1e-12

N_POOL_SUB = 22  # tiles whose subtract runs on GPSIMD (rest on DVE)

_CACHE = {}


def _build_nc(n=N, reps=1):
    import concourse.bass as bass
    import concourse.bacc as bacc
    import concourse.bass_isa as bass_isa
    import concourse.mybir as mybir
    from concourse.tile import TileContext

    Alu = mybir.AluOpType
    ACT = mybir.ActivationFunctionType
    dt = mybir.dt
    X = mybir.AxisListType.X
    XY = mybir.AxisListType.XY

    P = 128
    T = n // P
    TM = T * M
    NG = T // 4

    nc = bacc.Bacc()

    preds = nc.declare_dram_parameter("preds", [n, D_FEAT], dt.float32, isOutput=False)
    targets = nc.declare_dram_parameter("targets", [M, D_FEAT], dt.float32, isOutput=False)
    maskf = nc.declare_dram_parameter("maskf", [M], dt.float32, isOutput=False)
    out_assigned = nc.declare_dram_parameter("out_assigned", [n], dt.int32, isOutput=True)
    out_matched = nc.declare_dram_parameter("out_matched", [n], dt.int32, isOutput=True)

    scr_t2 = nc.dram_tensor("scr_t2", [M, K], dt.float32)
    scr_small = nc.dram_tensor("scr_small", [8, M], dt.float32)

    with TileContext(nc) as tc:
        with (
            tc.tile_pool(name="const", bufs=1) as cpool,
            tc.tile_pool(name="sb", bufs=1) as pool,
            tc.tile_pool(name="diffp", bufs=3) as diffp,
            tc.tile_pool(name="tmp", bufs=6) as tmp,
            tc.tile_pool(name="ps", bufs=2, space="PSUM") as psum,
            tc.tile_pool(name="pscx", bufs=1, space="PSUM") as pscx,
        ):
            f32 = dt.float32

            # ---------------- constants ----------------
            icol = cpool.tile([P, 1], f32)
            nc.gpsimd.iota(icol[:], pattern=[[0, 1]], channel_multiplier=1,
                           allow_small_or_imprecise_dtypes=True)
            irow = cpool.tile([P, P], f32)
            nc.gpsimd.iota(irow[:], pattern=[[1, P]], channel_multiplier=0,
                           allow_small_or_imprecise_dtypes=True)
            ident = cpool.tile([P, P], f32)
            nc.vector.tensor_scalar(ident[:], irow[:], icol[:], None, Alu.is_equal)
            bmi = cpool.tile([P, M], f32)
            nc.gpsimd.iota(bmi[:], pattern=[[-1, M]], base=int(BIG2),
                           channel_multiplier=0, allow_small_or_imprecise_dtypes=True)
            iota4 = cpool.tile([P, 4], f32)
            nc.gpsimd.iota(iota4[:], pattern=[[1, 4]], base=1, channel_multiplier=0,
                           allow_small_or_imprecise_dtypes=True)
            epsc = cpool.tile([P, 1], f32)
            nc.gpsimd.memset(epsc[:], EPS)

            def _body():
                # ---------------- input DMAs (targets first) ----------------
                T_sb = pool.tile([M, D_FEAT], f32)
                nc.sync.dma_start(out=T_sb[:], in_=targets[:])
                Mk = pool.tile([M, 1], f32)
                nc.sync.dma_start(out=Mk[:], in_=maskf[:].unsqueeze(1))

                # p-major: partition p holds rows n = p*T + t  (contiguous)
                P_sb = pool.tile([P, T * D_FEAT], f32)
                pview = preds[:].rearrange("(p t) d -> p t d", p=P)
                tch = max(1, T // 4)
                for t0 in range(0, T, tch):
                    t1 = min(T, t0 + tch)
                    nc.sync.dma_start(
                        out=P_sb[:].rearrange("p (t d) -> p t d", d=D_FEAT)[:, t0:t1],
                        in_=pview[:, t0:t1],
                    )
                Pv = P_sb[:].rearrange("p (t d) -> p t d", d=D_FEAT)

                # ---------------- target-side precompute (M partitions) -------
                tdx = T_sb[:, 6:78]
                v0 = pool.tile([M, K], f32)
                nc.vector.tensor_scalar(v0[:], tdx, 0.0, None, Alu.is_ge)
                v1 = pool.tile([M, K], f32)
                nc.vector.tensor_scalar(v1[:], tdx, IMG_W, None, Alu.is_lt)
                vv = pool.tile([M, K], f32)
                nc.vector.tensor_tensor(vv[:], v0[:], v1[:], Alu.mult)
                t2 = pool.tile([M, K], f32)
                nc.vector.tensor_tensor(t2[:], tdx, vv[:], Alu.mult)
                nc.sync.dma_start(out=scr_t2[:], in_=t2[:])
                TgtRep = pool.tile([P, M * K], f32)
                nc.sync.dma_start(
                    out=TgtRep[:],
                    in_=scr_t2[:].flatten().unsqueeze(0).broadcast_to([P, M * K]),
                )
                TgtRep3 = TgtRep[:].rearrange("p (m k) -> p m k", k=K)
                TgtRep4 = TgtRep[:].rearrange("p (u m k) -> p u m k", u=1, k=K)

                onemv = pool.tile([M, K], f32)
                nc.vector.tensor_scalar(onemv[:], vv[:], -1.0, 1.0, Alu.mult, Alu.add)
                lenr = pool.tile([M, 1], f32)
                nc.vector.tensor_reduce(lenr[:], vv[:], axis=X, op=Alu.add)
                lenc = pool.tile([M, 1], f32)
                nc.vector.tensor_scalar(lenc[:], lenr[:], 1.0, None, Alu.max)
                invlen = pool.tile([M, 1], f32)
                nc.vector.reciprocal(invlen[:], lenc[:])
                a30 = pool.tile([M, 1], f32)
                nc.vector.tensor_scalar(a30[:], lenr[:], 30.0, None, Alu.mult)
                aeps = pool.tile([M, 1], f32)
                nc.vector.tensor_scalar(aeps[:], a30[:], 1e-9, None, Alu.add)

                spack = pool.tile([M, 8], f32)
                nc.vector.tensor_copy(spack[:, 0:1], T_sb[:, 2:3])   # tx
                nc.vector.tensor_copy(spack[:, 1:2], T_sb[:, 3:4])   # ty
                nc.vector.tensor_copy(spack[:, 2:3], T_sb[:, 4:5])   # tth
                nc.vector.tensor_copy(spack[:, 3:4], T_sb[:, 1:2])   # label
                nc.vector.tensor_copy(spack[:, 4:5], invlen[:])
                nc.vector.tensor_copy(spack[:, 5:6], Mk[:])
                nc.vector.tensor_copy(spack[:, 6:7], Mk[:])
                nc.vector.tensor_copy(spack[:, 7:8], Mk[:])
                nc.sync.dma_start(out=scr_small[:].rearrange("i m -> m i"), in_=spack[:])
                SRep = pool.tile([P, 8 * M], f32)
                nc.sync.dma_start(
                    out=SRep[:],
                    in_=scr_small[:].flatten().unsqueeze(0).broadcast_to([P, 8 * M]),
                )

                def srep(i):
                    return SRep[:, i * M:(i + 1) * M].unsqueeze(1).broadcast_to([P, T, M])

                dxf = tmp.tile([P, TM], f32, tag="tm")
                dyf = tmp.tile([P, TM], f32, tag="tm")
                gxy_in = pool.tile([P, 2], f32)
                gxy_out = pool.tile([P, 2], f32)
                nginv2 = pool.tile([P, 2], f32)
                xys = tmp.tile([P, TM], f32, tag="tm")
                ths = tmp.tile([P, TM], f32, tag="tm")
                sig = pool.tile([P, T * 2], f32)
                qq = pool.tile([P, T * 2], f32)
                lp = pool.tile([P, T * 2], f32)
                lq = pool.tile([P, T * 2], f32)
                p2 = pool.tile([P, T * 2], f32)
                q2 = pool.tile([P, T * 2], f32)
                pos = pool.tile([P, T * 2], f32)
                neg = pool.tile([P, T * 2], f32)
                fdiff = pool.tile([P, T * 2], f32)
                ddt = pool.tile([P, T], f32)
                gd_in = pool.tile([P, 1], f32)

                nc.scalar.activation(sig[:].rearrange("p (t c) -> p t c", c=2),
                                     Pv[:, :, 0:2], ACT.Sigmoid)

                onemvT_ps = pscx.tile([K, M], f32, tag="onemvT")
                nc.tensor.transpose(onemvT_ps[:], onemv[:], ident[0:M, 0:M])
                onemvT = pool.tile([K, M], f32)
                nc.scalar.activation(onemvT[:], onemvT_ps[:], ACT.Copy)

                # predxT [72, n] via PE transposes
                predxT = pool.tile([K, n], f32)
                for g in range(NG):
                    tp = psum.tile([K, 4 * P], f32, tag="tp")
                    for j in range(4):
                        t = 4 * g + j
                        nc.tensor.transpose(tp[:, j * P:(j + 1) * P], Pv[:, t, 6:78],
                                            ident[:])
                    nc.scalar.activation(predxT[:, g * 4 * P:(g + 1) * 4 * P], tp[:],
                                         ACT.Copy)

                Cps = pscx.tile([P, TM], f32, tag="cps")
                for t in range(T):
                    nc.tensor.matmul(
                        Cps[:, t * M:(t + 1) * M],
                        predxT[:, t * P:(t + 1) * P], onemvT[:],
                        start=True, stop=True,
                    )

                # ---------------- persistent [P, TM] tiles ----------------
                Draw = pool.tile([P, TM], f32)
                Dm = pool.tile([P, TM], f32)
                DTn = pool.tile([M, n], f32)
                xyf = pool.tile([P, TM], f32)
                thf = pool.tile([P, TM], f32)
                cls = pool.tile([P, TM], f32)
                Ef = pool.tile([P, TM], f32)
                cost = pool.tile([P, TM], f32)
                match = pool.tile([P, TM], f32)
                dist = pool.tile([P, TM], f32)
                Dm3 = Dm[:].rearrange("p (t m) -> p t m", m=M)
                d3 = lambda ap: ap.rearrange("p (t m) -> p t m", m=M)

                def pcol(c):
                    return Pv[:, :, c].unsqueeze(2).broadcast_to([P, T, M])

                # ---------------- filler emission (interleaved) -----------

                def f_pool_subs():
                    nc.gpsimd.tensor_tensor(d3(dxf[:]), pcol(2), srep(0), Alu.subtract)
                    nc.gpsimd.tensor_tensor(d3(dyf[:]), pcol(3), srep(1), Alu.subtract)
                    nc.gpsimd.tensor_tensor(d3(thf[:]), pcol(4), srep(2), Alu.subtract)

                def f_squares():
                    nc.scalar.activation(dxf[:], dxf[:], ACT.Square)
                    nc.scalar.activation(dyf[:], dyf[:], ACT.Square)
                    nc.scalar.activation(thf[:], thf[:], ACT.Abs)

                def f_xyf():
                    nc.vector.tensor_tensor(xyf[:], dxf[:], dyf[:], Alu.add)
                    nc.scalar.activation(xyf[:], xyf[:], ACT.Sqrt)

                def f_qq():
                    nc.vector.tensor_scalar(qq[:], sig[:], -1.0, 1.0, Alu.mult, Alu.add)

                def f_lnln():
                    nc.scalar.activation(lp[:], sig[:], ACT.Ln, bias=epsc[:])
                    nc.scalar.activation(lq[:], qq[:], ACT.Ln, bias=epsc[:])

                def f_focal2():
                    nc.vector.tensor_tensor(p2[:], sig[:], sig[:], Alu.mult)
                    nc.vector.tensor_tensor(q2[:], qq[:], qq[:], Alu.mult)
                    nc.vector.scalar_tensor_tensor(pos[:], lp[:], -0.25, q2[:],
                                                   Alu.mult, Alu.mult)
                    nc.vector.scalar_tensor_tensor(neg[:], lq[:], -0.75, p2[:],
                                                   Alu.mult, Alu.mult)
                    nc.vector.tensor_tensor(fdiff[:], pos[:], neg[:], Alu.subtract)
                    fv = fdiff[:].rearrange("p (t c) -> p t c", c=2)
                    nc.vector.tensor_tensor(ddt[:], fv[:, :, 1], fv[:, :, 0],
                                            Alu.subtract)

                def f_cls():
                    fv = fdiff[:].rearrange("p (t c) -> p t c", c=2)
                    d0b = fv[:, :, 0].unsqueeze(2).broadcast_to([P, T, M])
                    ddb = ddt[:].unsqueeze(2).broadcast_to([P, T, M])
                    nc.gpsimd.tensor_tensor(d3(cls[:]), srep(3), ddb, Alu.mult)
                    nc.gpsimd.tensor_tensor(d3(cls[:]), d3(cls[:]), d0b, Alu.add)

                def f_gxy():
                    nc.vector.tensor_reduce(gxy_in[:, 0:1], d3(xyf[:]), axis=XY,
                                            op=Alu.max)
                    nc.vector.tensor_reduce(gxy_in[:, 1:2], d3(thf[:]), axis=XY,
                                            op=Alu.max)
                    nc.gpsimd.partition_all_reduce(gxy_out[:], gxy_in[:], channels=P,
                                                   reduce_op=bass_isa.ReduceOp.max)

                def f_nginv():
                    g2 = pool.tile([P, 2], f32)
                    nc.vector.tensor_scalar(g2[:], gxy_out[:], 1e-6, None, Alu.max)
                    gi = pool.tile([P, 2], f32)
                    nc.vector.reciprocal(gi[:], g2[:])
                    nc.vector.tensor_scalar(nginv2[:], gi[:], -1.0, None, Alu.mult)
                    nc.scalar.activation(xys[:], xyf[:], ACT.Copy, bias=1.01,
                                         scale=nginv2[:, 0:1])
                    nc.scalar.activation(ths[:], thf[:], ACT.Copy, bias=1.01,
                                         scale=nginv2[:, 1:2])

                def f_ef():
                    nc.gpsimd.tensor_tensor(Ef[:], xys[:], ths[:], Alu.mult)

                fillers = [f_pool_subs, f_qq, f_lnln, f_squares, f_focal2,
                           f_xyf, f_cls, f_gxy, f_nginv, f_ef]

                # ---------------- heavy loop ----------------
                pair_route = []
                npo = N_POOL_SUB // 2
                nd = T // 2 - npo
                a, b = npo, nd
                for i in range(T // 2):
                    if a > 0 and (b == 0 or a * nd >= b * npo or i % 2 == 0):
                        pair_route.append(True); a -= 1
                    else:
                        pair_route.append(False); b -= 1

                fi = 0
                for pi in range(T // 2):
                    t = 2 * pi
                    diff = diffp.tile([P, 2 * M * K], f32, tag="diff")
                    dv = diff[:].rearrange("p (u m k) -> p u m k", u=2, k=K)
                    if pair_route[pi]:
                        pb = Pv[:, t:t + 2, 6:78].unsqueeze(2).broadcast_to([P, 2, M, K])
                        tgtb = TgtRep4.broadcast_to([P, 2, M, K])
                        nc.gpsimd.tensor_tensor(dv, pb, tgtb, Alu.subtract)
                    else:
                        for u in range(2):
                            pbu = Pv[:, t + u, 6:78].unsqueeze(1).broadcast_to([P, M, K])
                            nc.vector.tensor_tensor(dv[:, u], pbu, TgtRep3,
                                                    Alu.subtract)
                    nc.vector.tensor_reduce(
                        Draw[:, t * M:(t + 2) * M].rearrange("p (u m) -> p u m", u=2),
                        dv, axis=X, op=Alu.add, apply_absolute_value=True,
                    )
                    if pi % 2 == 1:
                        g = pi // 2
                        sl = slice(g * 4 * M, (g + 1) * 4 * M)
                        nc.vector.tensor_tensor(Dm[:, sl], Draw[:, sl], Cps[:, sl],
                                                Alu.subtract)
                        # dist for this group + running per-partition max
                        nc.vector.tensor_tensor(
                            d3(dist[:])[:, 4 * g:4 * g + 4, :],
                            Dm3[:, 4 * g:4 * g + 4, :], srep(4)[:, 0:4, :], Alu.mult)
                        if g == 0:
                            nc.vector.tensor_reduce(
                                gd_in[:],
                                dist[:, sl].rearrange("p (a q) -> p a q", a=1),
                                axis=XY, op=Alu.max)
                        else:
                            gdp = pool.tile([P, 1], f32, tag="gdp")
                            nc.vector.tensor_reduce(
                                gdp[:],
                                dist[:, sl].rearrange("p (a q) -> p a q", a=1),
                                axis=XY, op=Alu.max)
                            nc.vector.tensor_tensor(gd_in[:], gd_in[:], gdp[:],
                                                    Alu.max)
                        tpd = psum.tile([M, 4 * P], f32, tag="tpT")
                        for j in range(4):
                            tt = 4 * g + j
                            nc.tensor.transpose(tpd[:, j * P:(j + 1) * P],
                                                Dm3[:, tt, :], ident[:])
                        nc.scalar.activation(DTn[:, g * 4 * P:(g + 1) * 4 * P],
                                             tpd[:], ACT.Copy, scale=-1.0)
                    if fi < len(fillers):
                        fillers[fi](); fi += 1
                while fi < len(fillers):
                    fillers[fi](); fi += 1

                # ---------------- d8 / iou / ks ----------------
                d8 = pool.tile([M, 8], f32)
                nc.vector.max(d8[:], DTn[:])
                num4 = pool.tile([M, 4], f32)
                nc.vector.tensor_scalar(num4[:], d8[:, 0:4], a30[:], None, Alu.add)
                den4 = pool.tile([M, 4], f32)
                nc.vector.tensor_scalar(den4[:], d8[:, 0:4], -1.0, aeps[:],
                                        Alu.mult, Alu.add)
                rec4 = pool.tile([M, 4], f32)
                nc.vector.reciprocal(rec4[:], den4[:])
                iou4 = pool.tile([M, 4], f32)
                nc.vector.tensor_tensor(iou4[:], num4[:], rec4[:], Alu.mult)
                nc.vector.tensor_scalar(iou4[:], iou4[:], Mk[:], 0.0, Alu.mult, Alu.max)
                S4 = pool.tile([M, 1], f32)
                nc.vector.tensor_reduce(S4[:], iou4[:], axis=X, op=Alu.add)
                ge2 = pool.tile([M, 1], f32)
                nc.vector.tensor_scalar(ge2[:], S4[:], 2.0, None, Alu.is_ge)
                ge3 = pool.tile([M, 1], f32)
                nc.vector.tensor_scalar(ge3[:], S4[:], 3.0, None, Alu.is_ge)
                ks = pool.tile([M, 1], f32)
                nc.vector.tensor_scalar(ks[:], S4[:], 4.0, None, Alu.is_ge)
                nc.vector.tensor_tensor(ks[:], ks[:], ge2[:], Alu.add)
                nc.vector.tensor_tensor(ks[:], ks[:], ge3[:], Alu.add)
                nc.vector.tensor_scalar(ks[:], ks[:], 1.0, None, Alu.add)
                e4 = pool.tile([M, 4], f32)
                nc.vector.tensor_scalar(e4[:], iota4[0:M, :], ks[:], None, Alu.is_equal)

                # ---------------- tail: cost ----------------
                gd_out = pool.tile([P, 1], f32)
                nc.gpsimd.partition_all_reduce(gd_out[:], gd_in[:], channels=P,
                                               reduce_op=bass_isa.ReduceOp.max)
                gd2 = pool.tile([P, 1], f32)
                nc.vector.tensor_scalar(gd2[:], gd_out[:], 1e-6, None, Alu.max)
                gdi = pool.tile([P, 1], f32)
                nc.vector.reciprocal(gdi[:], gd2[:])
                ngdi = pool.tile([P, 1], f32)
                nc.vector.tensor_scalar(ngdi[:], gdi[:], -1.0, None, Alu.mult)

                ds_ = tmp.tile([P, TM], f32, tag="tm")
                nc.scalar.activation(ds_[:], dist[:], ACT.Copy, bias=1.01,
                                     scale=ngdi[:])
                s3 = tmp.tile([P, TM], f32, tag="tm")
                nc.vector.tensor_tensor(s3[:], ds_[:], Ef[:], Alu.mult)
                sq3 = tmp.tile([P, TM], f32, tag="tm")
                nc.scalar.activation(sq3[:], s3[:], ACT.Square, scale=1.7320508)
                nc.vector.tensor_tensor(cost[:], cls[:], sq3[:], Alu.subtract)

                costTn = pool.tile([M, n], f32)
                cv = cost[:].rearrange("p (t m) -> p t m", m=M)
                for g in range(NG):
                    tpg = psum.tile([M, 4 * P], f32, tag="tpT")
                    for j in range(4):
                        t = 4 * g + j
                        nc.tensor.transpose(tpg[:, j * P:(j + 1) * P], cv[:, t, :],
                                            ident[:])
                    if g % 2 == 0:
                        nc.scalar.activation(costTn[:, g * 4 * P:(g + 1) * 4 * P],
                                             tpg[:], ACT.Copy, scale=-1.0)
                    else:
                        nc.vector.tensor_scalar(
                            costTn[:, g * 4 * P:(g + 1) * 4 * P], tpg[:],
                            -1.0, None, Alu.mult)

                c8 = pool.tile([M, 8], f32)
                nc.vector.max(c8[:], costTn[:])
                tn4 = pool.tile([M, 4], f32)
                nc.vector.tensor_tensor(tn4[:], c8[:, 0:4], e4[:], Alu.mult)
                thn = pool.tile([M, 1], f32)
                nc.vector.tensor_reduce(thn[:], tn4[:], axis=X, op=Alu.add)  # -thresh
                thn_e = pool.tile([M, 1], f32)
                nc.vector.tensor_scalar(thn_e[:], thn[:], Mk[:], None, Alu.mult)
                tmsk = pool.tile([M, 1], f32)
                nc.vector.tensor_scalar(tmsk[:], Mk[:], -BIGINF, BIGINF,
                                        Alu.mult, Alu.add)
                nc.vector.tensor_tensor(thn_e[:], thn_e[:], tmsk[:], Alu.add)
                thT_ps = pscx.tile([1, M], f32, tag="thT")
                nc.tensor.transpose(thT_ps[:], thn_e[:], ident[0:M, 0:M])
                thT = pool.tile([1, M], f32)
                nc.vector.tensor_scalar(thT[:], thT_ps[:], -1.0, None, Alu.mult)
                Thr = pool.tile([P, M], f32)
                nc.gpsimd.partition_broadcast(Thr[:], thT[:], channels=P)
                thrb = Thr[:].unsqueeze(1).broadcast_to([P, T, M])

                # ---------------- matching + conflict resolution ----------
                nc.vector.tensor_tensor(d3(match[:]), d3(cost[:]), thrb, Alu.is_le)
                mgt = pool.tile([P, T], f32)
                nc.vector.tensor_reduce(mgt[:], d3(match[:]), axis=X, op=Alu.add)

                bmib = bmi[:].unsqueeze(1).broadcast_to([P, T, M])
                pm1 = tmp.tile([P, TM], f32, tag="tm")
                nc.vector.scalar_tensor_tensor(d3(pm1[:]), d3(match[:]), 1.0, bmib,
                                               Alu.mult, Alu.mult)
                i1r = pool.tile([P, T], f32)
                nc.vector.tensor_reduce(i1r[:], d3(pm1[:]), axis=X, op=Alu.max)

                q1 = tmp.tile([P, TM], f32, tag="tm")
                nc.vector.tensor_scalar(q1[:], match[:], -BIGINF, BIGINF,
                                        Alu.mult, Alu.add)
                cm = tmp.tile([P, TM], f32, tag="tm")
                nc.vector.tensor_tensor(cm[:], cost[:], q1[:], Alu.add)
                mn2 = pool.tile([P, T], f32)
                nc.vector.tensor_reduce(mn2[:], d3(cm[:]), axis=X, op=Alu.min)
                mn2b = mn2[:].unsqueeze(2).broadcast_to([P, T, M])
                eq2 = tmp.tile([P, TM], f32, tag="tm")
                nc.vector.tensor_tensor(d3(eq2[:]), d3(cm[:]), mn2b, Alu.is_equal)
                eq2b = tmp.tile([P, TM], f32, tag="tm")
                nc.vector.scalar_tensor_tensor(d3(eq2b[:]), d3(eq2[:]), 1.0, bmib,
                                               Alu.mult, Alu.mult)
                i2r = pool.tile([P, T], f32)
                nc.vector.tensor_reduce(i2r[:], d3(eq2b[:]), axis=X, op=Alu.max)

                conf = pool.tile([P, T], f32)
                nc.vector.tensor_scalar(conf[:], mgt[:], 1.0, None, Alu.is_gt)
                asg = pool.tile([P, T], f32)
                nc.vector.tensor_scalar(asg[:], mgt[:], 0.0, None, Alu.is_gt)
                idx1 = pool.tile([P, T], f32)
                nc.vector.tensor_scalar(idx1[:], i1r[:], -1.0, BIG2, Alu.mult, Alu.add)
                idx2 = pool.tile([P, T], f32)
                nc.vector.tensor_scalar(idx2[:], i2r[:], -1.0, BIG2, Alu.mult, Alu.add)
                didx = pool.tile([P, T], f32)
                nc.vector.tensor_tensor(didx[:], idx2[:], idx1[:], Alu.subtract)
                nc.vector.tensor_tensor(didx[:], conf[:], didx[:], Alu.mult)
                mt = pool.tile([P, T], f32)
                nc.vector.tensor_tensor(mt[:], idx1[:], didx[:], Alu.add)
                nc.vector.tensor_tensor(mt[:], mt[:], asg[:], Alu.mult)
                nc.vector.tensor_tensor(mt[:], mt[:], asg[:], Alu.add)
                nc.vector.tensor_scalar(mt[:], mt[:], -1.0, None, Alu.add)

                asg_i = pool.tile([P, T], dt.int32)
                nc.vector.tensor_copy(asg_i[:], asg[:])
                mt_i = pool.tile([P, T], dt.int32)
                nc.vector.tensor_copy(mt_i[:], mt[:])
                # p-major: DRAM n = p*T + t -> per-partition contiguous runs
                nc.sync.dma_start(out=out_assigned[:].rearrange("(p t) -> p t", p=P),
                                  in_=asg_i[:])
                nc.sync.dma_start(out=out_matched[:].rearrange("(p t) -> p t", p=P),
                                  in_=mt_i[:])

            for _rep in range(reps):
                _body()
    nc.compile()
    return nc


def _get_nc(n=N, reps=1):
    key = (n, reps)
    if key not in _CACHE:
        _CACHE[key] = _build_nc(n, reps)
    return _CACHE[key]


def kernel(preds, targets, masks, img_w=800, img_h=320):
    from concourse.bass_utils import run_bass_kernel_spmd

    nc = _get_nc(N)
    preds = np.ascontiguousarray(preds, dtype=np.float32)
    targets = np.ascontiguousarray(targets, dtype=np.float32)
    maskf = np.ascontiguousarray(masks, dtype=np.float32)
    in_maps = [
        {"preds": preds[b], "targets": targets[b], "maskf": maskf[b]}
        for b in range(B)
    ]
    res = run_bass_kernel_spmd(nc, in_maps, list(range(B))).results
    assigned = np.stack([res[b]["out_assigned"] for b in range(B)]).astype(bool)
    matched = np.stack([res[b]["out_matched"] for b in range(B)]).astype(np.int32)
    return assigned, matched


# revision 8
# speedup vs baseline: 1.3224x; 1.0188x over previous
"""Trainium2 Bass kernel for CLRNet SimOTA-style assignment (B=8, N=4096, M=32, K=72).

Strategy: pure data-parallel over batch - one batch element per NeuronCore.

v4:
  * p-major n layout: partition p holds rows n = p*32 + t (contiguous DMA
    descriptors for preds load and output store).
  * Heavy stage: Pool subtract (2-tile batched) / DVE subtract (per tile),
    DVE abs-reduce (2-tile batched).  D = Draw - predx @ (1-v)^T (PE).
  * Small/medium work interleaved into the heavy loop's emission order.
  * Global maxes via gpsimd.partition_all_reduce; threshold broadcast via
    PE transpose + partition_broadcast; latency-critical tail on DVE only.
"""

import os
import sys

sys.path.insert(0, "/opt/trn_rl_repo")

import numpy as np

B, N, M, K = 8, 4096, 32, 72
D_FEAT = 78
IMG_W = 800.0
BIG = 100000.0
BIG2 = 100000.0
BIGINF = 1e30
EPS = 1e-12

N_POOL_SUB = 22  # tiles whose subtract runs on GPSIMD (rest on DVE)

_CACHE = {}


def _build_nc(n=N, reps=1):
    import concourse.bass as bass
    import concourse.bacc as bacc
    import concourse.bass_isa as bass_isa
    import concourse.mybir as mybir
    from concourse.tile import TileContext

    Alu = mybir.AluOpType
    ACT = mybir.ActivationFunctionType
    dt = mybir.dt
    X = mybir.AxisListType.X
    XY = mybir.AxisListType.XY

    P = 128
    T = n // P
    TM = T * M
    NG = T // 4

    nc = bacc.Bacc()

    preds = nc.declare_dram_parameter("preds", [n, D_FEAT], dt.float32, isOutput=False)
    targets = nc.declare_dram_parameter("targets", [M, D_FEAT], dt.float32, isOutput=False)
    maskf = nc.declare_dram_parameter("maskf", [M], dt.float32, isOutput=False)
    out_assigned = nc.declare_dram_parameter("out_assigned", [n], dt.int32, isOutput=True)
    out_matched = nc.declare_dram_parameter("out_matched", [n], dt.int32, isOutput=True)

    scr_t2 = nc.dram_tensor("scr_t2", [M, K], dt.float32)
    scr_small = nc.dram_tensor("scr_small", [8, M], dt.float32)

    with TileContext(nc) as tc:
        with (
            tc.tile_pool(name="const", bufs=1) as cpool,
            tc.tile_pool(name="sb", bufs=1) as pool,
            tc.tile_pool(name="diffp", bufs=4) as diffp,
            tc.tile_pool(name="tmp", bufs=6) as tmp,
            tc.tile_pool(name="ps", bufs=2, space="PSUM") as psum,
            tc.tile_pool(name="pscx", bufs=1, space="PSUM") as pscx,
        ):
            f32 = dt.float32

            # ---------------- constants (emitted lazily inside _body) ----------
            icol = cpool.tile([P, 1], f32)
            irow = cpool.tile([P, P], f32)
            ident = cpool.tile([P, P], f32)
            bmi = cpool.tile([P, M], f32)
            iota4 = cpool.tile([P, 4], f32)
            epsc = cpool.tile([P, 1], f32)
            consts_emitted = [False]

            def emit_consts():
                if consts_emitted[0]:
                    return
                consts_emitted[0] = True
                nc.gpsimd.iota(icol[:], pattern=[[0, 1]], channel_multiplier=1,
                               allow_small_or_imprecise_dtypes=True)
                nc.gpsimd.iota(irow[:], pattern=[[1, P]], channel_multiplier=0,
                               allow_small_or_imprecise_dtypes=True)
                nc.vector.tensor_scalar(ident[:], irow[:], icol[:], None, Alu.is_equal)
                nc.gpsimd.iota(bmi[:], pattern=[[-1, M]], base=int(BIG2),
                               channel_multiplier=0,
                               allow_small_or_imprecise_dtypes=True)
                nc.gpsimd.iota(iota4[:], pattern=[[1, 4]], base=1,
                               channel_multiplier=0,
                               allow_small_or_imprecise_dtypes=True)
                nc.gpsimd.memset(epsc[:], EPS)

            def _body():
                # ---------------- input DMAs (targets first) ----------------
                T_sb = pool.tile([M, D_FEAT], f32)
                nc.sync.dma_start(out=T_sb[:], in_=targets[:])
                Mk = pool.tile([M, 1], f32)
                nc.sync.dma_start(out=Mk[:], in_=maskf[:].unsqueeze(1))

                # p-major: partition p holds rows n = p*T + t  (contiguous)
                P_sb = pool.tile([P, T * D_FEAT], f32)
                pview = preds[:].rearrange("(p t) d -> p t d", p=P)
                tch = max(1, T // 4)
                for t0 in range(0, T, tch):
                    t1 = min(T, t0 + tch)
                    nc.sync.dma_start(
                        out=P_sb[:].rearrange("p (t d) -> p t d", d=D_FEAT)[:, t0:t1],
                        in_=pview[:, t0:t1],
                    )
                Pv = P_sb[:].rearrange("p (t d) -> p t d", d=D_FEAT)

                # ---------------- target-side precompute (M partitions) -------
                tdx = T_sb[:, 6:78]
                v0 = pool.tile([M, K], f32)
                nc.vector.tensor_scalar(v0[:], tdx, 0.0, None, Alu.is_ge)
                v1 = pool.tile([M, K], f32)
                nc.vector.tensor_scalar(v1[:], tdx, IMG_W, None, Alu.is_lt)
                vv = pool.tile([M, K], f32)
                nc.vector.tensor_tensor(vv[:], v0[:], v1[:], Alu.mult)
                t2 = pool.tile([M, K], f32)
                nc.vector.tensor_tensor(t2[:], tdx, vv[:], Alu.mult)
                nc.sync.dma_start(out=scr_t2[:], in_=t2[:])
                TgtRep = pool.tile([P, M * K], f32)
                nc.sync.dma_start(
                    out=TgtRep[:],
                    in_=scr_t2[:].flatten().unsqueeze(0).broadcast_to([P, M * K]),
                )
                emit_consts()
                TgtRep3 = TgtRep[:].rearrange("p (m k) -> p m k", k=K)
                TgtRep4 = TgtRep[:].rearrange("p (u m k) -> p u m k", u=1, k=K)

                onemv = pool.tile([M, K], f32)
                nc.vector.tensor_scalar(onemv[:], vv[:], -1.0, 1.0, Alu.mult, Alu.add)
                lenr = pool.tile([M, 1], f32)
                nc.vector.tensor_reduce(lenr[:], vv[:], axis=X, op=Alu.add)
                lenc = pool.tile([M, 1], f32)
                nc.vector.tensor_scalar(lenc[:], lenr[:], 1.0, None, Alu.max)
                invlen = pool.tile([M, 1], f32)
                nc.vector.reciprocal(invlen[:], lenc[:])
                a30 = pool.tile([M, 1], f32)
                nc.vector.tensor_scalar(a30[:], lenr[:], 30.0, None, Alu.mult)
                aeps = pool.tile([M, 1], f32)
                nc.vector.tensor_scalar(aeps[:], a30[:], 1e-9, None, Alu.add)

                spack = pool.tile([M, 8], f32)
                nc.vector.tensor_copy(spack[:, 0:1], T_sb[:, 2:3])   # tx
                nc.vector.tensor_copy(spack[:, 1:2], T_sb[:, 3:4])   # ty
                nc.vector.tensor_copy(spack[:, 2:3], T_sb[:, 4:5])   # tth
                nc.vector.tensor_copy(spack[:, 3:4], T_sb[:, 1:2])   # label
                nc.vector.tensor_copy(spack[:, 4:5], invlen[:])
                nc.vector.tensor_copy(spack[:, 5:6], Mk[:])
                nc.vector.tensor_copy(spack[:, 6:7], Mk[:])
                nc.vector.tensor_copy(spack[:, 7:8], Mk[:])
                nc.sync.dma_start(out=scr_small[:].rearrange("i m -> m i"), in_=spack[:])
                SRep = pool.tile([P, 8 * M], f32)
                nc.sync.dma_start(
                    out=SRep[:],
                    in_=scr_small[:].flatten().unsqueeze(0).broadcast_to([P, 8 * M]),
                )

                def srep(i):
                    return SRep[:, i * M:(i + 1) * M].unsqueeze(1).broadcast_to([P, T, M])

                dxf = tmp.tile([P, TM], f32, tag="tm")
                dyf = tmp.tile([P, TM], f32, tag="tm")
                gxy_in = pool.tile([P, 2], f32)
                gxy_out = pool.tile([P, 2], f32)
                nginv2 = pool.tile([P, 2], f32)
                xys = tmp.tile([P, TM], f32, tag="tm")
                ths = tmp.tile([P, TM], f32, tag="tm")
                sig = pool.tile([P, T * 2], f32)
                qq = pool.tile([P, T * 2], f32)
                lp = pool.tile([P, T * 2], f32)
                lq = pool.tile([P, T * 2], f32)
                p2 = pool.tile([P, T * 2], f32)
                q2 = pool.tile([P, T * 2], f32)
                pos = pool.tile([P, T * 2], f32)
                neg = pool.tile([P, T * 2], f32)
                fdiff = pool.tile([P, T * 2], f32)
                ddt = pool.tile([P, T], f32)
                gd_in = pool.tile([P, 1], f32)

                nc.scalar.activation(sig[:].rearrange("p (t c) -> p t c", c=2),
                                     Pv[:, :, 0:2], ACT.Sigmoid)

                onemvT_ps = pscx.tile([K, M], f32, tag="onemvT")
                nc.tensor.transpose(onemvT_ps[:], onemv[:], ident[0:M, 0:M])
                onemvT = pool.tile([K, M], f32)
                nc.scalar.activation(onemvT[:], onemvT_ps[:], ACT.Copy)

                # predxT [72, n] via PE transposes
                predxT = pool.tile([K, n], f32)
                for g in range(NG):
                    tp = psum.tile([K, 4 * P], f32, tag="tp")
                    for j in range(4):
                        t = 4 * g + j
                        nc.tensor.transpose(tp[:, j * P:(j + 1) * P], Pv[:, t, 6:78],
                                            ident[:])
                    nc.scalar.activation(predxT[:, g * 4 * P:(g + 1) * 4 * P], tp[:],
                                         ACT.Copy)

                Cps = pscx.tile([P, TM], f32, tag="cps")
                for t in range(T):
                    nc.tensor.matmul(
                        Cps[:, t * M:(t + 1) * M],
                        predxT[:, t * P:(t + 1) * P], onemvT[:],
                        start=True, stop=True,
                    )

                # ---------------- persistent [P, TM] tiles ----------------
                Draw = pool.tile([P, TM], f32)
                Dm = pool.tile([P, TM], f32)
                DTn = pool.tile([M, n], f32)
                xyf = pool.tile([P, TM], f32)
                thf = pool.tile([P, TM], f32)
                cls = pool.tile([P, TM], f32)
                Ef = pool.tile([P, TM], f32)
                cost = pool.tile([P, TM], f32)
                dist = pool.tile([P, TM], f32)
                Dm3 = Dm[:].rearrange("p (t m) -> p t m", m=M)
                d3 = lambda ap: ap.rearrange("p (t m) -> p t m", m=M)

                def pcol(c):
                    return Pv[:, :, c].unsqueeze(2).broadcast_to([P, T, M])

                # ---------------- filler emission (interleaved) -----------

                def f_pool_subs():
                    nc.gpsimd.tensor_tensor(d3(dxf[:]), pcol(2), srep(0), Alu.subtract)
                    nc.gpsimd.tensor_tensor(d3(dyf[:]), pcol(3), srep(1), Alu.subtract)
                    nc.gpsimd.tensor_tensor(d3(thf[:]), pcol(4), srep(2), Alu.subtract)

                def f_squares():
                    nc.scalar.activation(dxf[:], dxf[:], ACT.Square)
                    nc.scalar.activation(dyf[:], dyf[:], ACT.Square)
                    nc.scalar.activation(thf[:], thf[:], ACT.Abs)

                def f_xyf():
                    nc.vector.tensor_tensor(xyf[:], dxf[:], dyf[:], Alu.add)
                    nc.scalar.activation(xyf[:], xyf[:], ACT.Sqrt)

                def f_qq():
                    nc.vector.tensor_scalar(qq[:], sig[:], -1.0, 1.0, Alu.mult, Alu.add)

                def f_lnln():
                    nc.scalar.activation(lp[:], sig[:], ACT.Ln, bias=epsc[:])
                    nc.scalar.activation(lq[:], qq[:], ACT.Ln, bias=epsc[:])

                def f_focal2():
                    nc.vector.tensor_tensor(p2[:], sig[:], sig[:], Alu.mult)
                    nc.vector.tensor_tensor(q2[:], qq[:], qq[:], Alu.mult)
                    nc.vector.scalar_tensor_tensor(pos[:], lp[:], -0.25, q2[:],
                                                   Alu.mult, Alu.mult)
                    nc.vector.scalar_tensor_tensor(neg[:], lq[:], -0.75, p2[:],
                                                   Alu.mult, Alu.mult)
                    nc.vector.tensor_tensor(fdiff[:], pos[:], neg[:], Alu.subtract)
                    fv = fdiff[:].rearrange("p (t c) -> p t c", c=2)
                    nc.vector.tensor_tensor(ddt[:], fv[:, :, 1], fv[:, :, 0],
                                            Alu.subtract)

                def f_cls():
                    fv = fdiff[:].rearrange("p (t c) -> p t c", c=2)
                    d0b = fv[:, :, 0].unsqueeze(2).broadcast_to([P, T, M])
                    ddb = ddt[:].unsqueeze(2).broadcast_to([P, T, M])
                    nc.gpsimd.tensor_tensor(d3(cls[:]), srep(3), ddb, Alu.mult)
                    nc.gpsimd.tensor_tensor(d3(cls[:]), d3(cls[:]), d0b, Alu.add)

                def f_gxy():
                    nc.vector.tensor_reduce(gxy_in[:, 0:1], d3(xyf[:]), axis=XY,
                                            op=Alu.max)
                    nc.vector.tensor_reduce(gxy_in[:, 1:2], d3(thf[:]), axis=XY,
                                            op=Alu.max)
                    nc.gpsimd.partition_all_reduce(gxy_out[:], gxy_in[:], channels=P,
                                                   reduce_op=bass_isa.ReduceOp.max)

                def f_nginv():
                    g2 = pool.tile([P, 2], f32)
                    nc.vector.tensor_scalar(g2[:], gxy_out[:], 1e-6, None, Alu.max)
                    gi = pool.tile([P, 2], f32)
                    nc.vector.reciprocal(gi[:], g2[:])
                    nc.vector.tensor_scalar(nginv2[:], gi[:], -1.0, None, Alu.mult)
                    nc.scalar.activation(xys[:], xyf[:], ACT.Copy, bias=1.01,
                                         scale=nginv2[:, 0:1])
                    nc.scalar.activation(ths[:], thf[:], ACT.Copy, bias=1.01,
                                         scale=nginv2[:, 1:2])

                def f_ef():
                    nc.gpsimd.tensor_tensor(Ef[:], xys[:], ths[:], Alu.mult)

                fillers = [f_pool_subs, f_qq, f_lnln, f_squares, f_focal2,
                           f_xyf, f_cls, f_gxy, f_nginv, f_ef]

                # ---------------- heavy loop ----------------
                pair_route = []
                npo = N_POOL_SUB // 2
                nd = T // 2 - npo
                a, b = npo, nd
                for i in range(T // 2):
                    if a > 0 and (b == 0 or a * nd >= b * npo or i % 2 == 0):
                        pair_route.append(True); a -= 1
                    else:
                        pair_route.append(False); b -= 1

                fi = 0
                for pi in range(T // 2):
                    t = 2 * pi
                    diff = diffp.tile([P, 2 * M * K], f32, tag="diff")
                    dv = diff[:].rearrange("p (u m k) -> p u m k", u=2, k=K)
                    if pair_route[pi]:
                        pb = Pv[:, t:t + 2, 6:78].unsqueeze(2).broadcast_to([P, 2, M, K])
                        tgtb = TgtRep4.broadcast_to([P, 2, M, K])
                        nc.gpsimd.tensor_tensor(dv, pb, tgtb, Alu.subtract)
                    else:
                        for u in range(2):
                            pbu = Pv[:, t + u, 6:78].unsqueeze(1).broadcast_to([P, M, K])
                            nc.vector.tensor_tensor(dv[:, u], pbu, TgtRep3,
                                                    Alu.subtract)
                    nc.vector.tensor_reduce(
                        Draw[:, t * M:(t + 2) * M].rearrange("p (u m) -> p u m", u=2),
                        dv, axis=X, op=Alu.add, apply_absolute_value=True,
                    )
                    if pi % 2 == 1:
                        g = pi // 2
                        sl = slice(g * 4 * M, (g + 1) * 4 * M)
                        nc.vector.tensor_tensor(Dm[:, sl], Draw[:, sl], Cps[:, sl],
                                                Alu.subtract)
                        # dist for this group + running per-partition max
                        nc.vector.tensor_tensor(
                            d3(dist[:])[:, 4 * g:4 * g + 4, :],
                            Dm3[:, 4 * g:4 * g + 4, :], srep(4)[:, 0:4, :], Alu.mult)
                        if g == 0:
                            nc.vector.tensor_reduce(
                                gd_in[:],
                                dist[:, sl].rearrange("p (a q) -> p a q", a=1),
                                axis=XY, op=Alu.max)
                        else:
                            gdp = pool.tile([P, 1], f32, tag="gdp")
                            nc.vector.tensor_reduce(
                                gdp[:],
                                dist[:, sl].rearrange("p (a q) -> p a q", a=1),
                                axis=XY, op=Alu.max)
                            nc.vector.tensor_tensor(gd_in[:], gd_in[:], gdp[:],
                                                    Alu.max)
                        tpd = psum.tile([M, 4 * P], f32, tag="tpT")
                        for j in range(4):
                            tt = 4 * g + j
                            nc.tensor.transpose(tpd[:, j * P:(j + 1) * P],
                                                Dm3[:, tt, :], ident[:])
                        nc.scalar.activation(DTn[:, g * 4 * P:(g + 1) * 4 * P],
                                             tpd[:], ACT.Copy, scale=-1.0)
                    if fi < len(fillers):
                        fillers[fi](); fi += 1
                while fi < len(fillers):
                    fillers[fi](); fi += 1

                # ---------------- d8 / iou / ks ----------------
                d8 = pool.tile([M, 8], f32)
                nc.vector.max(d8[:], DTn[:])
                num4 = pool.tile([M, 4], f32)
                nc.vector.tensor_scalar(num4[:], d8[:, 0:4], a30[:], None, Alu.add)
                den4 = pool.tile([M, 4], f32)
                nc.vector.tensor_scalar(den4[:], d8[:, 0:4], -1.0, aeps[:],
                                        Alu.mult, Alu.add)
                rec4 = pool.tile([M, 4], f32)
                nc.vector.reciprocal(rec4[:], den4[:])
                iou4 = pool.tile([M, 4], f32)
                nc.vector.tensor_tensor(iou4[:], num4[:], rec4[:], Alu.mult)
                nc.vector.tensor_scalar(iou4[:], iou4[:], Mk[:], 0.0, Alu.mult, Alu.max)
                S4 = pool.tile([M, 1], f32)
                nc.vector.tensor_reduce(S4[:], iou4[:], axis=X, op=Alu.add)
                ge2 = pool.tile([M, 1], f32)
                nc.vector.tensor_scalar(ge2[:], S4[:], 2.0, None, Alu.is_ge)
                ge3 = pool.tile([M, 1], f32)
                nc.vector.tensor_scalar(ge3[:], S4[:], 3.0, None, Alu.is_ge)
                ks = pool.tile([M, 1], f32)
                nc.vector.tensor_scalar(ks[:], S4[:], 4.0, None, Alu.is_ge)
                nc.vector.tensor_tensor(ks[:], ks[:], ge2[:], Alu.add)
                nc.vector.tensor_tensor(ks[:], ks[:], ge3[:], Alu.add)
                nc.vector.tensor_scalar(ks[:], ks[:], 1.0, None, Alu.add)
                e4 = pool.tile([M, 4], f32)
                nc.vector.tensor_scalar(e4[:], iota4[0:M, :], ks[:], None, Alu.is_equal)

                # ---------------- tail: cost ----------------
                gd_out = pool.tile([P, 1], f32)
                nc.gpsimd.partition_all_reduce(gd_out[:], gd_in[:], channels=P,
                                               reduce_op=bass_isa.ReduceOp.max)
                gd2 = pool.tile([P, 1], f32)
                nc.vector.tensor_scalar(gd2[:], gd_out[:], 1e-6, None, Alu.max)
                gdi = pool.tile([P, 1], f32)
                nc.vector.reciprocal(gdi[:], gd2[:])
                ngdi = pool.tile([P, 1], f32)
                nc.vector.tensor_scalar(ngdi[:], gdi[:], -1.0, None, Alu.mult)

                ds_ = tmp.tile([P, TM], f32, tag="tm")
                nc.scalar.activation(ds_[:], dist[:], ACT.Copy, bias=1.01,
                                     scale=ngdi[:])
                s3 = tmp.tile([P, TM], f32, tag="tm")
                nc.vector.tensor_tensor(s3[:], ds_[:], Ef[:], Alu.mult)
                sq3 = tmp.tile([P, TM], f32, tag="tm")
                nc.scalar.activation(sq3[:], s3[:], ACT.Square, scale=1.7320508)
                nc.vector.tensor_tensor(cost[:], cls[:], sq3[:], Alu.subtract)

                costTn = pool.tile([M, n], f32)
                c8h = pool.tile([M, 16], f32)
                cv = cost[:].rearrange("p (t m) -> p t m", m=M)
                for g in range(NG):
                    tpg = psum.tile([M, 4 * P], f32, tag="tpT")
                    for j in range(4):
                        t = 4 * g + j
                        nc.tensor.transpose(tpg[:, j * P:(j + 1) * P], cv[:, t, :],
                                            ident[:])
                    if g % 2 == 0:
                        nc.scalar.activation(costTn[:, g * 4 * P:(g + 1) * 4 * P],
                                             tpg[:], ACT.Copy, scale=-1.0)
                    else:
                        nc.vector.tensor_scalar(
                            costTn[:, g * 4 * P:(g + 1) * 4 * P], tpg[:],
                            -1.0, None, Alu.mult)
                    if g == NG // 2 - 1:
                        nc.vector.max(c8h[:, 0:8], costTn[:, 0:n // 2])

                nc.vector.max(c8h[:, 8:16], costTn[:, n // 2:n])
                c8 = pool.tile([M, 8], f32)
                nc.vector.max(c8[:], c8h[:])
                tn4 = pool.tile([M, 4], f32)
                nc.vector.tensor_tensor(tn4[:], c8[:, 0:4], e4[:], Alu.mult)
                thn = pool.tile([M, 1], f32)
                nc.vector.tensor_reduce(thn[:], tn4[:], axis=X, op=Alu.add)  # -thresh
                thn_e = pool.tile([M, 1], f32)
                nc.vector.tensor_scalar(thn_e[:], thn[:], Mk[:], None, Alu.mult)
                tmsk = pool.tile([M, 1], f32)
                nc.vector.tensor_scalar(tmsk[:], Mk[:], -BIGINF, BIGINF,
                                        Alu.mult, Alu.add)
                nc.vector.tensor_tensor(thn_e[:], thn_e[:], tmsk[:], Alu.add)
                thT_ps = pscx.tile([1, M], f32, tag="thT")
                nc.tensor.transpose(thT_ps[:], thn_e[:], ident[0:M, 0:M])
                thT = pool.tile([1, M], f32)
                nc.vector.tensor_scalar(thT[:], thT_ps[:], -1.0, None, Alu.mult)
                Thr = pool.tile([P, M], f32)
                nc.gpsimd.partition_broadcast(Thr[:], thT[:], channels=P)
                thrb = Thr[:].unsqueeze(1).broadcast_to([P, T, M])

                # ---------------- matching + conflict resolution ----------
                match = tmp.tile([P, TM], f32, tag="tm")
                nc.vector.tensor_tensor(d3(match[:]), d3(cost[:]), thrb, Alu.is_le)
                mgt = pool.tile([P, T], f32)
                nc.vector.tensor_reduce(mgt[:], d3(match[:]), axis=X, op=Alu.add)

                bmib = bmi[:].unsqueeze(1).broadcast_to([P, T, M])
                pm1 = tmp.tile([P, TM], f32, tag="tm")
                nc.vector.scalar_tensor_tensor(d3(pm1[:]), d3(match[:]), 1.0, bmib,
                                               Alu.mult, Alu.mult)
                i1r = pool.tile([P, T], f32)
                nc.vector.tensor_reduce(i1r[:], d3(pm1[:]), axis=X, op=Alu.max)

                q1 = tmp.tile([P, TM], f32, tag="tm")
                nc.vector.tensor_scalar(q1[:], match[:], -BIGINF, BIGINF,
                                        Alu.mult, Alu.add)
                cm = tmp.tile([P, TM], f32, tag="tm")
                nc.vector.tensor_tensor(cm[:], cost[:], q1[:], Alu.add)
                mn2 = pool.tile([P, T], f32)
                nc.vector.tensor_reduce(mn2[:], d3(cm[:]), axis=X, op=Alu.min)
                mn2b = mn2[:].unsqueeze(2).broadcast_to([P, T, M])
                eq2 = tmp.tile([P, TM], f32, tag="tm")
                nc.vector.tensor_tensor(d3(eq2[:]), d3(cm[:]), mn2b, Alu.is_equal)
                eq2b = tmp.tile([P, TM], f32, tag="tm")
                nc.vector.scalar_tensor_tensor(d3(eq2b[:]), d3(eq2[:]), 1.0, bmib,
                                               Alu.mult, Alu.mult)
                i2r = pool.tile([P, T], f32)
                nc.vector.tensor_reduce(i2r[:], d3(eq2b[:]), axis=X, op=Alu.max)

                conf = pool.tile([P, T], f32)
                nc.vector.tensor_scalar(conf[:], mgt[:], 1.0, None, Alu.is_gt)
                asg = pool.tile([P, T], f32)
                nc.vector.tensor_scalar(asg[:], mgt[:], 0.0, None, Alu.is_gt)
                idx1 = pool.tile([P, T], f32)
                nc.vector.tensor_scalar(idx1[:], i1r[:], -1.0, BIG2, Alu.mult, Alu.add)
                idx2 = pool.tile([P, T], f32)
                nc.vector.tensor_scalar(idx2[:], i2r[:], -1.0, BIG2, Alu.mult, Alu.add)
                didx = pool.tile([P, T], f32)
                nc.vector.tensor_tensor(didx[:], idx2[:], idx1[:], Alu.subtract)
                nc.vector.tensor_tensor(didx[:], conf[:], didx[:], Alu.mult)
                mt = pool.tile([P, T], f32)
                nc.vector.tensor_tensor(mt[:], idx1[:], didx[:], Alu.add)
                nc.vector.tensor_tensor(mt[:], mt[:], asg[:], Alu.mult)
                nc.vector.tensor_tensor(mt[:], mt[:], asg[:], Alu.add)
                nc.vector.tensor_scalar(mt[:], mt[:], -1.0, None, Alu.add)

                asg_i = pool.tile([P, T], dt.int32)
                nc.vector.tensor_copy(asg_i[:], asg[:])
                mt_i = pool.tile([P, T], dt.int32)
                nc.vector.tensor_copy(mt_i[:], mt[:])
                # p-major: DRAM n = p*T + t -> per-partition contiguous runs
                nc.sync.dma_start(out=out_assigned[:].rearrange("(p t) -> p t", p=P),
                                  in_=asg_i[:])
                nc.sync.dma_start(out=out_matched[:].rearrange("(p t) -> p t", p=P),
                                  in_=mt_i[:])

            for _rep in range(reps):
                _body()
    nc.compile()
    return nc


def _get_nc(n=N, reps=1):
    key = (n, reps)
    if key not in _CACHE:
        _CACHE[key] = _build_nc(n, reps)
    return _CACHE[key]


def kernel(preds, targets, masks, img_w=800, img_h=320):
    from concourse.bass_utils import run_bass_kernel_spmd

    nc = _get_nc(N)
    preds = np.ascontiguousarray(preds, dtype=np.float32)
    targets = np.ascontiguousarray(targets, dtype=np.float32)
    maskf = np.ascontiguousarray(masks, dtype=np.float32)
    in_maps = [
        {"preds": preds[b], "targets": targets[b], "maskf": maskf[b]}
        for b in range(B)
    ]
    res = run_bass_kernel_spmd(nc, in_maps, list(range(B))).results
    assigned = np.stack([res[b]["out_assigned"] for b in range(B)]).astype(bool)
    matched = np.stack([res[b]["out_matched"] for b in range(B)]).astype(np.int32)
    return assigned, matched
